# revision 36
# baseline (speedup 1.0000x reference)
"""DGCNN_Grouper (knn + edgeconv + mlps) Trainium2 Bass kernel.

Strategy: batch-parallel over 8 NeuronCores (B=8 -> 1 sample/core).
Per core (N=2048, K=16):
  - knn: D(n,m) computed as ONE augmented K=6 f32r matmul per (128,512)
    chunk: D = [x,1,sq,1]^T . [-2x, sq, 1, 1].  Top-16-smallest per row
    via bit-packed keys: key = bits(D+? no: D)^0x7fffffff with low 11
    mantissa bits replaced by the column index, then vector max8 /
    match_replace / max8 (exact set selection, index rides in low bits).
  - edge features:  y[n,k,:] = u[idx[n,k]] + v[n]  where
       u = h @ W1a^T, v = h @ (W1b-W1a)^T, h = f^T W_it^T + b_it
    (u,v folded to direct-from-f matmuls). u gathered row-wise from DRAM
    with the gpsimd dma_gather custom op, PE-transposed into
    (channel, edge) layout, v added via an accumulated 0/1 matmul.
  - GroupNorm stats from per-channel accum_out sums; affine+leakyrelu as
    z + relu(-0.8 z) with scale/bias folded into the ACT op.
  - mlp1/maxes fully on chip, global maxes folded with relu/bias.
  - mlp2/mlp3 on (n) with the gmax contribution folded into the bias.
"""

import os
import sys

import numpy as np

for _p in ("/opt/trn_rl_repo", "/root/.axon_site/_ro/trn_rl_repo"):
    if os.path.isdir(_p) and _p not in sys.path:
        sys.path.insert(0, _p)

import concourse.bacc as bacc
import concourse.bass as bass
import concourse.tile as tile
from concourse import library_config, mybir
from concourse.bass_utils import run_bass_kernel_spmd

FP = mybir.dt.float32
FR = mybir.dt.float32r if os.environ.get("USE_F32R", "1") == "1" else mybir.dt.float32
I32 = mybir.dt.int32
I16 = mybir.dt.int16

N = 2048
K = 16
GROUPS = 4
EPS = 1e-5
NCHUNK = N // 512  # 4
NTILES = N // 128  # 16
NEDGE = N * K  # 32768
ECHUNKS = NEDGE // 512  # 64

AX = mybir.AxisListType
ALU = mybir.AluOpType
ACTF = mybir.ActivationFunctionType

INPUT_SPECS = [
    ("x", (3, N)), ("f", (3, N)),
    ("W_it", (64, 3)), ("b_it", (64,)),
    ("W1", (128, 128)), ("gn_g", (128,)), ("gn_b", (128,)),
    ("Wa", (256, 128)), ("ba", (256,)),
    ("Wb", (512, 256)), ("bb", (512,)),
    ("Wc", (256, 640)), ("bc", (256,)),
    ("Wd", (128, 256)), ("bd", (128,)),
    ("We", (256, 128)), ("be", (256,)),
    ("Wf", (512, 256)), ("bf", (512,)),
]


def _ts(i, sz):
    return slice(i * sz, (i + 1) * sz)


def build_program():
    nc = bacc.Bacc("TRN2", target_bir_lowering=False, debug=False)

    ins = {}
    for name, shape in INPUT_SPECS:
        ins[name] = nc.dram_tensor(name, list(shape), FP, kind="ExternalInput").ap()
    glob_out = nc.dram_tensor("glob", [512, 1], FP, kind="ExternalOutput").ap()
    loc_out = nc.dram_tensor("loc", [128, N], FP, kind="ExternalOutput").ap()
    u_rows = nc.dram_tensor("u_rows", [N, 128], FP).ap()
    dbg = {}
    if os.environ.get("DEBUG_OUT", "0") == "1":
        dbg["idxT"] = nc.dram_tensor("dbg_idxT", [16, N], I16, kind="ExternalOutput").ap()
        dbg["lbraw"] = nc.dram_tensor("dbg_lbraw", [128, N], FP, kind="ExternalOutput").ap()
        dbg["ysum"] = nc.dram_tensor("dbg_ysum", [128, ECHUNKS], FP, kind="ExternalOutput").ap()
        dbg["ysq"] = nc.dram_tensor("dbg_ysq", [128, ECHUNKS], FP, kind="ExternalOutput").ap()
        dbg["gmaxr"] = nc.dram_tensor("dbg_gmaxr", [128, 4], FP, kind="ExternalOutput").ap()
        dbg["urows"] = nc.dram_tensor("dbg_urows", [N, 128], FP, kind="ExternalOutput").ap()
        dbg["y0"] = nc.dram_tensor("dbg_y0", [128, 512], FP, kind="ExternalOutput").ap()
        dbg["ug0"] = nc.dram_tensor("dbg_ug0", [128, 8 * 128], FP, kind="ExternalOutput").ap()
        dbg["lb"] = nc.dram_tensor("dbg_lb", [128, N], FP, kind="ExternalOutput").ap()

    # constant data baked into the NEFF
    ident_np = np.eye(128, dtype=np.float32)
    ident_t = nc.inline_tensor(ident_np, name="ident").ap()
    ident16_t = nc.inline_tensor(np.eye(128, dtype=np.int16), name="ident16").ap()
    iota_np = np.broadcast_to(np.arange(N, dtype=np.int32), (128, N)).copy()
    iota_t = nc.inline_tensor(iota_np, name="iotat").ap()
    # B pattern repeated every 32 partitions so a v-slice at base partition
    # 32*j can pair with rhs slice b32[32*j:32*(j+1), :] (matmul requires
    # equal base partitions for lhsT and rhs).
    b32_np = np.zeros((128, 512), dtype=np.float32)
    for p in range(128):
        r = p % 32
        b32_np[p, r * 16:(r + 1) * 16] = 1.0
    b32_t = nc.inline_tensor(b32_np, name="b32").ap()
    ones_t_ap = nc.inline_tensor(np.ones((1, N), dtype=np.float32), name="onesrow").ap()
    zeros16_t = nc.inline_tensor(np.zeros((128, N), dtype=np.int16), name="zeros16").ap()
    gi_np = np.zeros((128, GROUPS), dtype=np.float32)
    for g in range(GROUPS):
        gi_np[g * 32:(g + 1) * 32, g] = 1.0
    gi_t = nc.inline_tensor(gi_np, name="gi").ap()
    git_t = nc.inline_tensor(gi_np.T.copy(), name="git").ap()

    with tile.TileContext(nc) as tc:
        _body(nc, tc, ins, glob_out, loc_out, u_rows,
              ident_t, ident16_t, iota_t, b32_t, gi_t, git_t, ones_t_ap,
              zeros16_t, dbg)
    nc.compile()
    return nc


def _body(nc, tc, ins, glob_out, loc_out, u_rows,
          ident_t, ident16_t, iota_t, b32_t, gi_t, git_t, ones_t, zeros16_t,
          dbg=None):
    dbg = dbg or {}
    from contextlib import ExitStack

    ctx = ExitStack()
    with ctx:
        wp = ctx.enter_context(tc.tile_pool(name="wp", bufs=1))
        pctx = ExitStack()
        pp = pctx.enter_context(tc.tile_pool(name="pp", bufs=1))
        wpsum = pctx.enter_context(tc.tile_pool(name="wpsum", bufs=2, space="PSUM"))

        # ---------- constants into SBUF ----------
        ident = wp.tile([128, 128], FP, tag="ident")
        nc.sync.dma_start(ident[:], ident_t[:])
        ident16 = pp.tile([128, 128], I16, tag="ident16")
        nc.sync.dma_start(ident16[:], ident16_t[:])
        b32f = pp.tile([128, 512], FP, tag="b32f")
        nc.sync.dma_start(b32f[:], b32_t[:])
        b32 = wp.tile([128, 512], FR, tag="b32")
        nc.vector.tensor_copy(b32[:], b32f[:])
        gi = wp.tile([128, GROUPS], FP, tag="gi")
        nc.sync.dma_start(gi[:], gi_t[:])
        git = wp.tile([GROUPS, 128], FP, tag="git")
        nc.sync.dma_start(git[:], git_t[:])

        # ---------- load weights + transposes ----------
        def load_nat(name, O, C):
            """W (O,C) -> sbuf tile (128, (O//128)*C); block j holds rows
            j*128..j*128+127."""
            ap = ins[name]
            jb = O // 128
            t = pp.tile([128, jb * C], FP, tag=f"nat_{name}")
            for j in range(jb):
                nc.sync.dma_start(t[:, _ts(j, C)], ap[_ts(j, 128), :])
            return t

        def transpose_weight(name, O, C, dtype=FP):
            """Return list of tiles T[k] (128, O) with T[k][c,o] = W[o, k*128+c].
            dtype=FR makes the DVE psum->sbuf copy emit fp32r (rounded) so the
            tile can feed fp32r matmuls."""
            nat = load_nat(name, O, C)
            jb, kb = O // 128, C // 128
            tiles = []
            for k in range(kb):
                tk = wp.tile([128, O], dtype, tag=f"wT_{name}_{k}")
                for j in range(jb):
                    ps = wpsum.tile([128, 128], FP, tag="wtps")
                    nc.tensor.matmul(
                        ps[:],
                        lhsT=nat[:, _ts(j, C)][:, _ts(k, 128)],
                        rhs=ident[:],
                        is_transpose=True, skip_group_check=True,
                    )
                    nc.vector.tensor_copy(tk[:, _ts(j, 128)], ps[:])
                tiles.append(tk)
            return tiles

        def load_vec_cols(name, D, tag=None):
            """(D,) -> (128, D//128): col c = elems c*128..c*128+127."""
            cb = max(1, D // 128)
            p = min(D, 128)
            t = wp.tile([p, cb], FP, tag=tag or f"vec_{name}")
            ap = ins[name]
            for c in range(cb):
                nc.sync.dma_start(t[:, c:c + 1], ap[_ts(c, p), None])
            return t

        waT = transpose_weight("Wa", 256, 128, FR)[0]   # (128, 256)
        wbT = transpose_weight("Wb", 512, 256, FR)      # 2 x (128, 512)
        wcT = transpose_weight("Wc", 256, 640)          # 5 x (128, 256)
        wdT = transpose_weight("Wd", 128, 256, FR)      # 2 x (128, 128)
        weT = transpose_weight("We", 256, 128)[0]       # (128, 256)
        wfT = transpose_weight("Wf", 512, 256, FR)      # 2 x (128, 512)

        ba_c = load_vec_cols("ba", 256)
        bb_c = load_vec_cols("bb", 512)
        bc_c = load_vec_cols("bc", 256)
        bd_c = load_vec_cols("bd", 128)
        be_c = load_vec_cols("be", 256)
        bf_c = load_vec_cols("bf", 512)
        gng = load_vec_cols("gn_g", 128)
        gnb = load_vec_cols("gn_b", 128)
        bit = load_vec_cols("b_it", 64)  # wp, tiny

        # W1 -> W1T, split a/b, w1bm = W1b - W1a
        w1nat = load_nat("W1", 128, 128)
        w1T = pp.tile([128, 128], FP, tag="w1T")
        ps = wpsum.tile([128, 128], FP, tag="wtps")
        nc.tensor.matmul(ps[:], lhsT=w1nat[:], rhs=ident[:],
                         is_transpose=True, skip_group_check=True)
        nc.vector.tensor_copy(w1T[:], ps[:])
        w1aT = w1T[0:64, :]     # (64, 128)
        # shift W1T rows 64:128 to base partition 0 (DVE needs equal base
        # partitions when both operands are in SBUF)
        w1bT0 = pp.tile([64, 128], FP, tag="w1bT0")
        nc.sync.dma_start(w1bT0[:], w1T[64:128, :])
        w1bmT = pp.tile([64, 128], FP, tag="w1bm")
        nc.vector.tensor_tensor(out=w1bmT[:], in0=w1bT0[:], in1=w1aT,
                                op=ALU.subtract)

        w_it = pp.tile([64, 3], FP, tag="w_it")
        nc.sync.dma_start(w_it[:], ins["W_it"][:, :])

        # ---------- FL = [f; 1]  (4, N) ; x, sq rows ----------
        fl = pp.tile([4, N], FP, tag="fl")
        nc.sync.dma_start(fl[0:3, :], ins["f"][:, :])
        nc.sync.dma_start(fl[3:4, :], ones_t[:, :])

        xs = pp.tile([3, N], FP, tag="xs")
        nc.sync.dma_start(xs[:], ins["x"][:, :])
        xsq = pp.tile([3, N], FP, tag="xsq")
        nc.vector.tensor_tensor(out=xsq[:], in0=xs[:], in1=xs[:], op=ALU.mult)
        ones3 = pp.tile([3, 1], FP, tag="ones3")
        nc.sync.dma_start(ones3[:], ones_t[0:1, 0:3])

        # L = [x,1,sq,1] (6,N) lhsT ; R = [-2x, sq, 1, 1] (6,N) rhs
        lmat = pp.tile([6, N], FP, tag="lmat")
        rmat = pp.tile([6, N], FP, tag="rmat")
        nc.sync.dma_start(lmat[0:3, :], ins["x"][:, :])
        nc.sync.dma_start(lmat[3:4, :], ones_t[:, :])
        nc.sync.dma_start(lmat[5:6, :], ones_t[:, :])
        nc.scalar.mul(rmat[0:3, :], xs[:], -2.0)
        nc.sync.dma_start(rmat[4:5, :], ones_t[:, :])
        nc.sync.dma_start(rmat[5:6, :], ones_t[:, :])
        sqrow = pp.tile([1, N], FP, tag="sqrow")
        for j in range(NCHUNK):
            ps = wpsum.tile([1, 512], FP, tag="wtps")
            nc.tensor.matmul(ps[:], lhsT=ones3[:], rhs=xsq[:, _ts(j, 512)])
            nc.scalar.copy(sqrow[:, _ts(j, 512)], ps[:])
        nc.sync.dma_start(lmat[4:5, :], sqrow[:])
        nc.sync.dma_start(rmat[3:4, :], sqrow[:])

        # ---------- UR / VR (4,128): u = FL^T @ UR, v = FL^T @ VR ----------
        ur = pp.tile([4, 128], FP, tag="ur")
        vr = pp.tile([4, 128], FP, tag="vr")
        for dst, wT in ((ur, w1aT), (vr, w1bmT[:])):
            # rows 0:3 = W_it^T @ wT^T ... comb[i,o] = sum_c W_it[c,i] wT[c,o]
            ps = wpsum.tile([3, 128], FP, tag="wtps")
            nc.tensor.matmul(ps[:], lhsT=w_it[:], rhs=wT)
            nc.scalar.copy(dst[0:3, :], ps[:])
            # row 3 = b_it @ wT
            ps2 = wpsum.tile([1, 128], FP, tag="wtps")
            nc.tensor.matmul(ps2[:], lhsT=bit[:], rhs=wT)
            cst = pp.tile([1, 128], FP, tag="cstrow")
            nc.scalar.copy(cst[:], ps2[:])
            nc.sync.dma_start(dst[3:4, :], cst[:])

        # ---------- u rows -> DRAM ; v -> SBUF ----------
        # v stored pair-interleaved: chunk g (32 n-rows) lives at partition
        # base (g%2)*32, column block g//2 -- matmul lhsT/rhs base partitions
        # can only be 0/32/64.
        vctx = ExitStack()
        vpool = vctx.enter_context(tc.tile_pool(name="vpool", bufs=1, side="right"))
        v2 = vpool.tile([64, 32 * 128], FR, tag="v2")
        with tc.tile_pool(name="uvp", bufs=3) as uvp, \
                tc.tile_pool(name="uvps", bufs=3, space="PSUM") as uvpsum:
            for i in range(NTILES):
                pu = uvpsum.tile([128, 128], FP, tag="pu")
                nc.tensor.matmul(pu[:], lhsT=fl[:, _ts(i, 128)], rhs=ur[:])
                su = uvp.tile([128, 128], FP, tag="su")
                nc.scalar.copy(su[:], pu[:])
                nc.sync.dma_start(u_rows[_ts(i, 128), :], su[:])
                if dbg:
                    nc.sync.dma_start(dbg["urows"][_ts(i, 128), :], su[:])
            for m in range(32):
                pv = uvpsum.tile([64, 128], FP, tag="pv")
                for half in range(2):
                    g = 2 * m + half
                    nc.tensor.matmul(pv[_ts(half, 32), :],
                                     lhsT=fl[:, _ts(g, 32)], rhs=vr[:],
                                     skip_group_check=True)
                nc.scalar.copy(v2[:, _ts(m, 128)], pv[:])

        # ---------- knn ----------
        # 128-partition layout: dma_gather reads its index operand as a
        # (128, num_idxs/16) view using only the first 16 partitions, but
        # bounds-checks all of it -- zero the rest.
        idxT = wp.tile([128, N], I16, tag="idxT")
        nc.sync.dma_start(idxT[:], zeros16_t[:])
        with tc.tile_pool(name="knn", bufs=2) as kp, \
                tc.tile_pool(name="knnps", bufs=1, space="PSUM") as kps, \
                tc.tile_pool(name="knnpt", bufs=2, space="PSUM") as kpt, \
                tc.tile_pool(name="knnsm", bufs=2) as ksm:
            for t in range(NTILES):
                dp = kps.tile([128, N], FP, tag="dp")
                for j in range(NCHUNK):
                    nc.tensor.matmul(
                        dp[:, _ts(j, 512)],
                        lhsT=lmat[:, _ts(t, 128)],
                        rhs=rmat[:, _ts(j, 512)],
                    )
                keys = kp.tile([128, N], FP, tag="keys")
                ki = keys[:].bitcast(I32)
                # key = bits(D+1) ^ 0x7fffffff: positive normal floats,
                # strictly decreasing in D (the +1 rides in the matmul)
                nc.vector.tensor_scalar(
                    out=ki, in0=dp[:].bitcast(I32),
                    scalar1=0x7FFFFFFF, scalar2=None,
                    op0=ALU.bitwise_xor,
                )
                m16 = ksm.tile([128, 16], FP, tag="m16")
                idx16 = ksm.tile([128, 16], mybir.dt.uint16, tag="idx16")
                nc.vector.max(m16[:, 0:8], keys[:])
                nc.vector.max_index(idx16[:, 0:8], m16[:, 0:8], keys[:])
                nc.vector.match_replace(out=keys[:], in_to_replace=m16[:, 0:8],
                                        in_values=keys[:], imm_value=0.0)
                nc.vector.max(m16[:, 8:16], keys[:])
                nc.vector.max_index(idx16[:, 8:16], m16[:, 8:16], keys[:])
                # transpose (128,16) -> (16,128): go through fp32 VALUES
                # (0..2047 exact) -- int16 is not a legal PE transpose dtype
                idxf = ksm.tile([128, 16], FP, tag="idxf")
                nc.vector.tensor_copy(idxf[:], idx16[:])
                pt = kpt.tile([16, 128], FP, tag="pt")
                nc.tensor.matmul(pt[:], lhsT=idxf[:], rhs=ident[:],
                                 is_transpose=True, skip_group_check=True)
                nc.vector.tensor_copy(idxT[0:16, _ts(t, 128)], pt[:])

        # dma_gather reads its index operand per-Q7-core: each of the 8
        # cores reads its own 16-partition group, so the (16, N) index block
        # must be replicated across all 128 partitions.
        for r in range(7):
            nc.sync.dma_start(idxT[16 * (r + 1):16 * (r + 2), :], idxT[0:16, :])

        # ---------- pass 1: y = uT + v, stats ----------
        pctx.close()  # frees prep/knn-era SBUF before the big y buffer
        bigp = ctx.enter_context(tc.tile_pool(name="bigp", bufs=1))
        y_all = bigp.tile([128, NEDGE], FP, tag="y_all")
        ysum_c = wp.tile([128, ECHUNKS], FP, tag="ysum_c")
        ysq_c = wp.tile([128, ECHUNKS], FP, tag="ysq_c")
        with tc.tile_pool(name="gat", bufs=2) as gp, \
                tc.tile_pool(name="yps", bufs=2, space="PSUM") as yps, \
                tc.tile_pool(name="ysq", bufs=1) as ysqp:
            for gt in range(NTILES * 2):
                ug = gp.tile([128, 8, 128], FP, tag="ug")
                nc.gpsimd.dma_gather(
                    out_ap=ug[:], in_ap=u_rows[:],
                    idxs_ap=idxT[:, _ts(gt, 64)],
                    num_idxs=1024, num_idxs_reg=1024, elem_size=128,
                )
                if dbg and gt == 0:
                    nc.sync.dma_start(dbg["ug0"][:, :], ug[:].rearrange("p a b -> p (a b)"))
                for j in range(2):
                    c = gt * 2 + j
                    yp = yps.tile([128, 512], FP, tag="yp")
                    for q in range(4):
                        # start=True only on the first op: a start re-arms the
                        # whole 2KB lazy-zero region, clobbering sibling
                        # quarters already written.
                        nc.tensor.matmul(
                            yp[:, _ts(q, 128)],
                            lhsT=ug[:, j * 4 + q, :],
                            rhs=ident[:],
                            is_transpose=True, start=(q == 0), stop=False,
                            skip_group_check=True,
                        )
                    nc.tensor.matmul(
                        yp[:],
                        lhsT=v2[_ts(c % 2, 32), _ts(c // 2, 128)],
                        rhs=b32[_ts(c % 2, 32), :],
                        start=False, stop=True, skip_group_check=True,
                    )
                    nc.scalar.activation(
                        out=y_all[:, _ts(c, 512)], in_=yp[:], func=ACTF.Copy,
                        accum_out=ysum_c[:, c:c + 1],
                    )
                    sqs = ysqp.tile([128, 512], FP, tag="sqs")
                    nc.scalar.activation(
                        out=sqs[:], in_=yp[:], func=ACTF.Square,
                        accum_out=ysq_c[:, c:c + 1],
                    )

        if dbg:
            nc.sync.dma_start(dbg["idxT"][:, :], idxT[0:16, :])
            nc.sync.dma_start(dbg["ysum"][:, :], ysum_c[:])
            nc.sync.dma_start(dbg["ysq"][:, :], ysq_c[:])
            nc.sync.dma_start(dbg["y0"][:, :], y_all[:, 0:512])
        vctx.close()  # v2 dead after pass 1

        # ---------- group norm scale/bias ----------
        sc_sb = wp.tile([128, 1], FP, tag="sc_sb")
        tb_sb = wp.tile([128, 1], FP, tag="tb_sb")
        qs_sb = wp.tile([128, 1], FP, tag="qs_sb")
        qb_sb = wp.tile([128, 1], FP, tag="qb_sb")
        with tc.tile_pool(name="st", bufs=1) as stp, \
                tc.tile_pool(name="stps", bufs=1, space="PSUM") as stps:
            stat2 = stp.tile([128, 2], FP, tag="stat2")
            nc.vector.reduce_sum(stat2[:, 0:1], ysum_c[:], axis=AX.X)
            nc.vector.reduce_sum(stat2[:, 1:2], ysq_c[:], axis=AX.X)
            gps = stps.tile([GROUPS, 2], FP, tag="gps")
            nc.tensor.matmul(gps[:], lhsT=gi[:], rhs=stat2[:])
            cnt = float(NEDGE * 32)
            g2s = stp.tile([GROUPS, 2], FP, tag="g2s")
            nc.scalar.mul(g2s[:], gps[:], 1.0 / cnt)  # [mean, E[y^2]]
            rm = stp.tile([GROUPS, 2], FP, tag="rm")
            # rm[:,1] = mu ; rm[:,0] = rstd
            nc.scalar.copy(rm[:, 1:2], g2s[:, 0:1])
            musq = stp.tile([GROUPS, 1], FP, tag="musq")
            nc.scalar.square(musq[:], g2s[:, 0:1])
            var = stp.tile([GROUPS, 1], FP, tag="var")
            nc.vector.tensor_tensor(out=var[:], in0=g2s[:, 1:2], in1=musq[:],
                                    op=ALU.subtract)
            vpe = stp.tile([GROUPS, 1], FP, tag="vpe")
            nc.vector.tensor_scalar(out=vpe[:], in0=var[:], scalar1=EPS,
                                    scalar2=None, op0=ALU.add)
            vin = stp.tile([GROUPS, 1], FP, tag="vin")
            nc.vector.reciprocal(vin[:], vpe[:])
            nc.scalar.sqrt(rm[:, 0:1], vin[:])
            bc2p = stps.tile([128, 2], FP, tag="bc2p")
            nc.tensor.matmul(bc2p[:], lhsT=git[:], rhs=rm[:])
            bc2 = stp.tile([128, 2], FP, tag="bc2")
            nc.scalar.copy(bc2[:], bc2p[:])
            # sc = gn_g * rstd_c ; tb = gn_b - mu_c * sc
            nc.vector.tensor_tensor(out=sc_sb[:], in0=gng[:], in1=bc2[:, 0:1],
                                    op=ALU.mult)
            tmp = stp.tile([128, 1], FP, tag="tmp")
            nc.vector.tensor_tensor(out=tmp[:], in0=bc2[:, 1:2], in1=sc_sb[:],
                                    op=ALU.mult)
            nc.vector.tensor_tensor(out=tb_sb[:], in0=gnb[:], in1=tmp[:],
                                    op=ALU.subtract)
            nc.scalar.mul(qs_sb[:], sc_sb[:], -0.8)
            nc.scalar.mul(qb_sb[:], tb_sb[:], -0.8)

        # ---------- pass 2: ynorm -> g1 -> g2 -> maxes ----------
        # local_base comes from raw y (affine+leakyrelu commute with max_k
        # for gamma > 0), applied once at the end -- keeps the local path in
        # exact fp32 while the global path rides fp32r matmuls.
        lbraw = wp.tile([128, N], FP, tag="lbraw")
        lb = wp.tile([128, N], FP, tag="lb")
        gmc = wp.tile([128, 4 * ECHUNKS], FP, tag="gmc")  # 4 h-blocks of 64
        with tc.tile_pool(name="p2", bufs=2) as p2, \
                tc.tile_pool(name="g1ps", bufs=2, space="PSUM") as g1ps, \
                tc.tile_pool(name="g2ps", bufs=1, space="PSUM") as g2ps:
            for c in range(ECHUNKS):
                ysl = y_all[:, _ts(c, 512)]
                nc.vector.tensor_reduce(
                    out=lbraw[:, _ts(c, 32)],
                    in_=ysl.rearrange("p (n k) -> p n k", k=16),
                    op=ALU.max, axis=AX.X,
                )
                z = p2.tile([128, 512], FP, tag="z")
                nc.vector.tensor_scalar(out=z[:], in0=ysl, scalar1=sc_sb[:],
                                        scalar2=tb_sb[:], op0=ALU.mult,
                                        op1=ALU.add)
                q = p2.tile([128, 512], FP, tag="q")
                nc.scalar.activation(out=q[:], in_=ysl, func=ACTF.Relu,
                                     bias=qb_sb[:], scale=qs_sb[:])
                yn = p2.tile([128, 512], FR, tag="yn")
                nc.vector.tensor_tensor(out=yn[:], in0=z[:], in1=q[:],
                                        op=ALU.add)
                g1p = g1ps.tile([128, 1024], FP, tag="g1p")
                for h in range(2):
                    nc.tensor.matmul(g1p[:, _ts(h, 512)],
                                     lhsT=waT[:, _ts(h, 128)],
                                     rhs=yn[:])
                g1r = p2.tile([128, 1024], FR, tag="g1r")
                for h in range(2):
                    nc.scalar.activation(out=g1r[:, _ts(h, 512)],
                                         in_=g1p[:, _ts(h, 512)],
                                         func=ACTF.Relu, bias=ba_c[:, h:h + 1])
                g2p = g2ps.tile([128, 2048], FP, tag="g2p")
                for h in range(4):
                    for kk in range(2):
                        nc.tensor.matmul(
                            g2p[:, _ts(h, 512)],
                            lhsT=wbT[kk][:, _ts(h, 128)],
                            rhs=g1r[:, _ts(kk, 512)],
                            start=(kk == 0), stop=(kk == 1),
                        )
                for h in range(4):
                    nc.vector.tensor_reduce(
                        out=gmc[:, h * ECHUNKS + c:h * ECHUNKS + c + 1],
                        in_=g2p[:, _ts(h, 512)], op=ALU.max, axis=AX.X)

            # lb = leakyrelu(sc * lbraw + tb) = z + relu(-0.8 z), chunked
            for j in range(NCHUNK):
                lsl = lbraw[:, _ts(j, 512)]
                z2 = p2.tile([128, 512], FP, tag="z")
                nc.vector.tensor_scalar(out=z2[:], in0=lsl, scalar1=sc_sb[:],
                                        scalar2=tb_sb[:], op0=ALU.mult,
                                        op1=ALU.add)
                q2 = p2.tile([128, 512], FP, tag="q")
                nc.scalar.activation(out=q2[:], in_=lsl, func=ACTF.Relu,
                                     bias=qb_sb[:], scale=qs_sb[:])
                nc.vector.tensor_tensor(out=lb[:, _ts(j, 512)], in0=z2[:],
                                        in1=q2[:], op=ALU.add)

        if dbg:
            nc.sync.dma_start(dbg["lbraw"][:, :], lbraw[:])
            nc.sync.dma_start(dbg["lb"][:, :], lb[:])

        # ---------- gmax -> relu(+bb) ----------
        gmaxr = wp.tile([128, 4], FP, tag="gmaxr")
        with tc.tile_pool(name="gm", bufs=1) as gmp:
            for h in range(4):
                gm1 = gmp.tile([128, 1], FP, tag="gm1")
                nc.vector.tensor_reduce(out=gm1[:],
                                        in_=gmc[:, _ts(h, ECHUNKS)],
                                        op=ALU.max, axis=AX.X)
                nc.scalar.activation(out=gmaxr[:, h:h + 1], in_=gm1[:],
                                     func=ACTF.Relu, bias=bb_c[:, h:h + 1])

        if dbg:
            nc.sync.dma_start(dbg["gmaxr"][:, :], gmaxr[:])

        # ---------- tail: mlp2 / mlp3 ----------
        lf_sb = wp.tile([128, N], FP, tag="lf_sb")
        glcol = wp.tile([128, 4 * NCHUNK], FP, tag="glcol")
        with tc.tile_pool(name="tl", bufs=2) as tl, \
                tc.tile_pool(name="tlps", bufs=1, space="PSUM") as tlps, \
                tc.tile_pool(name="tlps2", bufs=1, space="PSUM") as tlps2:
            # wbias = bc + Wc[:,128:] @ gmaxr
            wbias = wp.tile([128, 2], FP, tag="wbias")
            for h in range(2):
                wcp = tlps.tile([128, 1], FP, tag="wcp")
                for k in range(1, 5):
                    nc.tensor.matmul(wcp[:],
                                     lhsT=wcT[k][:, _ts(h, 128)],
                                     rhs=gmaxr[:, k - 1:k],
                                     start=(k == 1), stop=(k == 4))
                nc.vector.tensor_tensor(out=wbias[:, h:h + 1], in0=wcp[:],
                                        in1=bc_c[:, h:h + 1], op=ALU.add)

            for j in range(NCHUNK):
                lf1p = tlps.tile([128, 1024], FP, tag="sh1024")
                for h in range(2):
                    nc.tensor.matmul(lf1p[:, _ts(h, 512)],
                                     lhsT=wcT[0][:, _ts(h, 128)],
                                     rhs=lb[:, _ts(j, 512)])
                lf1r = tl.tile([128, 1024], FR, tag="lf1r")
                for h in range(2):
                    nc.scalar.activation(out=lf1r[:, _ts(h, 512)],
                                         in_=lf1p[:, _ts(h, 512)],
                                         func=ACTF.Relu, bias=wbias[:, h:h + 1])
                lf2p = tlps2.tile([128, 512], FP, tag="lf2p")
                for kk in range(2):
                    nc.tensor.matmul(lf2p[:], lhsT=wdT[kk][:],
                                     rhs=lf1r[:, _ts(kk, 512)],
                                     start=(kk == 0), stop=(kk == 1))
                nc.scalar.activation(out=lf_sb[:, _ts(j, 512)], in_=lf2p[:],
                                     func=ACTF.Relu, bias=bd_c[:])
                gl1p = tlps.tile([128, 1024], FP, tag="sh1024")
                for h in range(2):
                    nc.tensor.matmul(gl1p[:, _ts(h, 512)],
                                     lhsT=weT[:, _ts(h, 128)],
                                     rhs=lf_sb[:, _ts(j, 512)])
                gl1r = tl.tile([128, 1024], FR, tag="gl1r")
                for h in range(2):
                    nc.scalar.activation(out=gl1r[:, _ts(h, 512)],
                                         in_=gl1p[:, _ts(h, 512)],
                                         func=ACTF.Relu, bias=be_c[:, h:h + 1])
                gl2p = tlps2.tile([128, 2048], FP, tag="gl2p")
                for h in range(4):
                    for kk in range(2):
                        nc.tensor.matmul(
                            gl2p[:, _ts(h, 512)],
                            lhsT=wfT[kk][:, _ts(h, 128)],
                            rhs=gl1r[:, _ts(kk, 512)],
                            start=(kk == 0), stop=(kk == 1))
                for h in range(4):
                    nc.vector.tensor_reduce(
                        out=glcol[:, h * NCHUNK + j:h * NCHUNK + j + 1],
                        in_=gl2p[:, _ts(h, 512)], op=ALU.max, axis=AX.X)

            nc.sync.dma_start(loc_out[:, :], lf_sb[:])
            for h in range(4):
                gf = tl.tile([128, 1], FP, tag="gf")
                nc.vector.tensor_reduce(out=gf[:], in_=glcol[:, _ts(h, NCHUNK)],
                                        op=ALU.max, axis=AX.X)
                gfr = tl.tile([128, 1], FP, tag="gfr")
                nc.scalar.activation(out=gfr[:], in_=gf[:], func=ACTF.Relu,
                                     bias=bf_c[:, h:h + 1])
                nc.sync.dma_start(glob_out[_ts(h, 128), :], gfr[:])


_NC_CACHE = None


def _get_program():
    global _NC_CACHE
    if _NC_CACHE is None:
        _NC_CACHE = build_program()
    return _NC_CACHE


def _make_in_maps(inputs):
    B = inputs["x"].shape[0]
    in_maps = []
    for b in range(B):
        m = {}
        for name, shape in INPUT_SPECS:
            arr = np.asarray(inputs[name], dtype=np.float32)
            if name in ("x", "f"):
                arr = arr[b]
            m[name] = np.ascontiguousarray(arr)
        in_maps.append(m)
    return in_maps


def kernel(**inputs):
    nc = _get_program()
    B = inputs["x"].shape[0]
    assert B == 8
    in_maps = _make_in_maps(inputs)
    res = run_bass_kernel_spmd(nc, in_maps, list(range(B)))
    glob = np.stack([res.results[b]["glob"] for b in range(B)])  # (B,512,1)
    loc = np.stack([res.results[b]["loc"] for b in range(B)])    # (B,128,N)
    return (glob.astype(np.float32), loc.astype(np.float32))


if __name__ == "__main__":
    prog = build_program()
    print("program built OK")


# revision 41
# speedup vs baseline: 453.4645x; 453.4645x over previous
"""DGCNN_Grouper (knn + edgeconv + mlps) Trainium2 Bass kernel.

Strategy: batch-parallel over 8 NeuronCores (B=8 -> 1 sample/core).
Per core (N=2048, K=16):
  - knn: D+1 computed as ONE augmented K=6 matmul per (128,512) chunk:
    D+1 = [x,1,sq,1]^T . [-2x, sq, 1, 1]. Top-16-smallest per row via
    keys = bits(D+1)^0x7fffffff (positive floats, descending in D) with
    vector max8 / max_index / match_replace / max8 / max_index --
    exact fp32 set selection.
  - edge features:  y[n,k,:] = u[idx[n,k]] + v[n]  where
       u = h @ W1a^T, v = h @ (W1b-W1a)^T, h = f^T W_it^T + b_it
    (u,v folded to direct-from-f matmuls). u gathered row-wise from DRAM
    with the gpsimd dma_gather custom op, PE-transposed into
    (channel, edge) layout, v added via an accumulated 0/1 matmul.
  - GroupNorm stats from per-channel accum_out sums; affine+leakyrelu as
    z + relu(-0.8 z) with scale/bias folded into the ACT op.
  - mlp1/maxes fully on chip, global maxes folded with relu/bias.
  - mlp2/mlp3 on (n) with the gmax contribution folded into the bias.
"""

import os
import sys

import numpy as np

for _p in ("/opt/trn_rl_repo", "/root/.axon_site/_ro/trn_rl_repo"):
    if os.path.isdir(_p) and _p not in sys.path:
        sys.path.insert(0, _p)

import concourse.bacc as bacc
import concourse.bass as bass
import concourse.tile as tile
from concourse import library_config, mybir
from concourse.bass_utils import run_bass_kernel_spmd

FP = mybir.dt.float32
FR = mybir.dt.float32r if os.environ.get("USE_F32R", "1") == "1" else mybir.dt.float32
I32 = mybir.dt.int32
I16 = mybir.dt.int16

N = 2048
K = 16
GROUPS = 4
EPS = 1e-5
NCHUNK = N // 512  # 4
NTILES = N // 128  # 16
NEDGE = N * K  # 32768
ECHUNKS = NEDGE // 512  # 64

AX = mybir.AxisListType
ALU = mybir.AluOpType
ACTF = mybir.ActivationFunctionType

INPUT_SPECS = [
    ("x", (3, N)), ("f", (3, N)),
    ("W_it", (64, 3)), ("b_it", (64,)),
    ("W1", (128, 128)), ("gn_g", (128,)), ("gn_b", (128,)),
    ("Wa", (256, 128)), ("ba", (256,)),
    ("Wb", (512, 256)), ("bb", (512,)),
    ("Wc", (256, 640)), ("bc", (256,)),
    ("Wd", (128, 256)), ("bd", (128,)),
    ("We", (256, 128)), ("be", (256,)),
    ("Wf", (512, 256)), ("bf", (512,)),
]


def _ts(i, sz):
    return slice(i * sz, (i + 1) * sz)


def build_program():
    nc = bacc.Bacc("TRN2", target_bir_lowering=False, debug=False)

    ins = {}
    for name, shape in INPUT_SPECS:
        ins[name] = nc.dram_tensor(name, list(shape), FP, kind="ExternalInput").ap()
    glob_out = nc.dram_tensor("glob", [512, 1], FP, kind="ExternalOutput").ap()
    loc_out = nc.dram_tensor("loc", [128, N], FP, kind="ExternalOutput").ap()
    u_rows = nc.dram_tensor("u_rows", [N, 128], FP).ap()
    dbg = {}
    if os.environ.get("DEBUG_OUT", "0") == "1":
        dbg["idxT"] = nc.dram_tensor("dbg_idxT", [16, N], I16, kind="ExternalOutput").ap()
        dbg["lbraw"] = nc.dram_tensor("dbg_lbraw", [128, N], FP, kind="ExternalOutput").ap()
        dbg["ysum"] = nc.dram_tensor("dbg_ysum", [128, ECHUNKS], FP, kind="ExternalOutput").ap()
        dbg["ysq"] = nc.dram_tensor("dbg_ysq", [128, ECHUNKS], FP, kind="ExternalOutput").ap()
        dbg["gmaxr"] = nc.dram_tensor("dbg_gmaxr", [128, 4], FP, kind="ExternalOutput").ap()
        dbg["urows"] = nc.dram_tensor("dbg_urows", [N, 128], FP, kind="ExternalOutput").ap()
        dbg["y0"] = nc.dram_tensor("dbg_y0", [128, 512], FP, kind="ExternalOutput").ap()
        dbg["ug0"] = nc.dram_tensor("dbg_ug0", [128, 8 * 128], FP, kind="ExternalOutput").ap()
        dbg["lb"] = nc.dram_tensor("dbg_lb", [128, N], FP, kind="ExternalOutput").ap()

    # constant data baked into the NEFF
    ident_np = np.eye(128, dtype=np.float32)
    ident_t = nc.inline_tensor(ident_np, name="ident").ap()
    ident16_t = nc.inline_tensor(np.eye(128, dtype=np.int16), name="ident16").ap()
    iota_np = np.broadcast_to(np.arange(N, dtype=np.int32), (128, N)).copy()
    iota_t = nc.inline_tensor(iota_np, name="iotat").ap()
    # B pattern repeated every 32 partitions so a v-slice at base partition
    # 32*j can pair with rhs slice b32[32*j:32*(j+1), :] (matmul requires
    # equal base partitions for lhsT and rhs).
    b32_np = np.zeros((128, 512), dtype=np.float32)
    for p in range(128):
        r = p % 32
        b32_np[p, r * 16:(r + 1) * 16] = 1.0
    b32_t = nc.inline_tensor(b32_np, name="b32").ap()
    ones_t_ap = nc.inline_tensor(np.ones((1, N), dtype=np.float32), name="onesrow").ap()
    zeros16_t = nc.inline_tensor(np.zeros((128, N), dtype=np.int16), name="zeros16").ap()
    gi_np = np.zeros((128, GROUPS), dtype=np.float32)
    for g in range(GROUPS):
        gi_np[g * 32:(g + 1) * 32, g] = 1.0
    gi_t = nc.inline_tensor(gi_np, name="gi").ap()
    git_t = nc.inline_tensor(gi_np.T.copy(), name="git").ap()

    with tile.TileContext(nc) as tc:
        _body(nc, tc, ins, glob_out, loc_out, u_rows,
              ident_t, ident16_t, iota_t, b32_t, gi_t, git_t, ones_t_ap,
              zeros16_t, dbg)
    nc.compile()
    return nc


def _body(nc, tc, ins, glob_out, loc_out, u_rows,
          ident_t, ident16_t, iota_t, b32_t, gi_t, git_t, ones_t, zeros16_t,
          dbg=None):
    dbg = dbg or {}
    from contextlib import ExitStack

    ctx = ExitStack()
    with ctx:
        wp = ctx.enter_context(tc.tile_pool(name="wp", bufs=1))
        pctx = ExitStack()
        pp = pctx.enter_context(tc.tile_pool(name="pp", bufs=1))
        wpsum = pctx.enter_context(tc.tile_pool(name="wpsum", bufs=2, space="PSUM"))

        # ---------- constants into SBUF ----------
        ident = wp.tile([128, 128], FP, tag="ident")
        nc.sync.dma_start(ident[:], ident_t[:])
        ident16 = pp.tile([128, 128], I16, tag="ident16")
        nc.sync.dma_start(ident16[:], ident16_t[:])
        b32f = pp.tile([128, 512], FP, tag="b32f")
        nc.sync.dma_start(b32f[:], b32_t[:])
        b32 = wp.tile([128, 512], FR, tag="b32")
        nc.vector.tensor_copy(b32[:], b32f[:])
        gi = wp.tile([128, GROUPS], FP, tag="gi")
        nc.sync.dma_start(gi[:], gi_t[:])
        git = wp.tile([GROUPS, 128], FP, tag="git")
        nc.sync.dma_start(git[:], git_t[:])

        # ---------- load weights + transposes ----------
        def load_nat(name, O, C):
            """W (O,C) -> sbuf tile (128, (O//128)*C); block j holds rows
            j*128..j*128+127."""
            ap = ins[name]
            jb = O // 128
            t = pp.tile([128, jb * C], FP, tag=f"nat_{name}")
            for j in range(jb):
                nc.sync.dma_start(t[:, _ts(j, C)], ap[_ts(j, 128), :])
            return t

        def transpose_weight(name, O, C, dtype=FP):
            """Return list of tiles T[k] (128, O) with T[k][c,o] = W[o, k*128+c].
            dtype=FR makes the DVE psum->sbuf copy emit fp32r (rounded) so the
            tile can feed fp32r matmuls."""
            nat = load_nat(name, O, C)
            jb, kb = O // 128, C // 128
            tiles = []
            for k in range(kb):
                tk = wp.tile([128, O], dtype, tag=f"wT_{name}_{k}")
                for j in range(jb):
                    ps = wpsum.tile([128, 128], FP, tag="wtps")
                    nc.tensor.matmul(
                        ps[:],
                        lhsT=nat[:, _ts(j, C)][:, _ts(k, 128)],
                        rhs=ident[:],
                        is_transpose=True, skip_group_check=True,
                    )
                    nc.vector.tensor_copy(tk[:, _ts(j, 128)], ps[:])
                tiles.append(tk)
            return tiles

        def load_vec_cols(name, D, tag=None):
            """(D,) -> (128, D//128): col c = elems c*128..c*128+127."""
            cb = max(1, D // 128)
            p = min(D, 128)
            t = wp.tile([p, cb], FP, tag=tag or f"vec_{name}")
            ap = ins[name]
            for c in range(cb):
                nc.sync.dma_start(t[:, c:c + 1], ap[_ts(c, p), None])
            return t

        waT = transpose_weight("Wa", 256, 128, FR)[0]   # (128, 256)
        wbT = transpose_weight("Wb", 512, 256, FR)      # 2 x (128, 512)
        wcT = transpose_weight("Wc", 256, 640)          # 5 x (128, 256)
        wdT = transpose_weight("Wd", 128, 256, FR)      # 2 x (128, 128)
        weT = transpose_weight("We", 256, 128)[0]       # (128, 256)
        wfT = transpose_weight("Wf", 512, 256, FR)      # 2 x (128, 512)

        ba_c = load_vec_cols("ba", 256)
        bb_c = load_vec_cols("bb", 512)
        bc_c = load_vec_cols("bc", 256)
        bd_c = load_vec_cols("bd", 128)
        be_c = load_vec_cols("be", 256)
        bf_c = load_vec_cols("bf", 512)
        gng = load_vec_cols("gn_g", 128)
        gnb = load_vec_cols("gn_b", 128)
        bit = load_vec_cols("b_it", 64)  # wp, tiny

        # W1 -> W1T, split a/b, w1bm = W1b - W1a
        w1nat = load_nat("W1", 128, 128)
        w1T = pp.tile([128, 128], FP, tag="w1T")
        ps = wpsum.tile([128, 128], FP, tag="wtps")
        nc.tensor.matmul(ps[:], lhsT=w1nat[:], rhs=ident[:],
                         is_transpose=True, skip_group_check=True)
        nc.vector.tensor_copy(w1T[:], ps[:])
        w1aT = w1T[0:64, :]     # (64, 128)
        # shift W1T rows 64:128 to base partition 0 (DVE needs equal base
        # partitions when both operands are in SBUF)
        w1bT0 = pp.tile([64, 128], FP, tag="w1bT0")
        nc.sync.dma_start(w1bT0[:], w1T[64:128, :])
        w1bmT = pp.tile([64, 128], FP, tag="w1bm")
        nc.vector.tensor_tensor(out=w1bmT[:], in0=w1bT0[:], in1=w1aT,
                                op=ALU.subtract)

        w_it = pp.tile([64, 3], FP, tag="w_it")
        nc.sync.dma_start(w_it[:], ins["W_it"][:, :])

        # ---------- FL = [f; 1]  (4, N) ; x, sq rows ----------
        fl = pp.tile([4, N], FP, tag="fl")
        nc.sync.dma_start(fl[0:3, :], ins["f"][:, :])
        nc.sync.dma_start(fl[3:4, :], ones_t[:, :])

        xs = pp.tile([3, N], FP, tag="xs")
        nc.sync.dma_start(xs[:], ins["x"][:, :])
        xsq = pp.tile([3, N], FP, tag="xsq")
        nc.vector.tensor_tensor(out=xsq[:], in0=xs[:], in1=xs[:], op=ALU.mult)
        ones3 = pp.tile([3, 1], FP, tag="ones3")
        nc.sync.dma_start(ones3[:], ones_t[0:1, 0:3])

        # L = [x,1,sq,1] (6,N) lhsT ; R = [-2x, sq, 1, 1] (6,N) rhs
        lmat = pp.tile([6, N], FP, tag="lmat")
        rmat = pp.tile([6, N], FP, tag="rmat")
        nc.sync.dma_start(lmat[0:3, :], ins["x"][:, :])
        nc.sync.dma_start(lmat[3:4, :], ones_t[:, :])
        nc.sync.dma_start(lmat[5:6, :], ones_t[:, :])
        nc.scalar.mul(rmat[0:3, :], xs[:], -2.0)
        nc.sync.dma_start(rmat[4:5, :], ones_t[:, :])
        nc.sync.dma_start(rmat[5:6, :], ones_t[:, :])
        sqrow = pp.tile([1, N], FP, tag="sqrow")
        for j in range(NCHUNK):
            ps = wpsum.tile([1, 512], FP, tag="wtps")
            nc.tensor.matmul(ps[:], lhsT=ones3[:], rhs=xsq[:, _ts(j, 512)])
            nc.scalar.copy(sqrow[:, _ts(j, 512)], ps[:])
        nc.sync.dma_start(lmat[4:5, :], sqrow[:])
        nc.sync.dma_start(rmat[3:4, :], sqrow[:])

        # ---------- UR / VR (4,128): u = FL^T @ UR, v = FL^T @ VR ----------
        ur = pp.tile([4, 128], FP, tag="ur")
        vr = pp.tile([4, 128], FP, tag="vr")
        for dst, wT in ((ur, w1aT), (vr, w1bmT[:])):
            # rows 0:3 = W_it^T @ wT^T ... comb[i,o] = sum_c W_it[c,i] wT[c,o]
            ps = wpsum.tile([3, 128], FP, tag="wtps")
            nc.tensor.matmul(ps[:], lhsT=w_it[:], rhs=wT)
            nc.scalar.copy(dst[0:3, :], ps[:])
            # row 3 = b_it @ wT
            ps2 = wpsum.tile([1, 128], FP, tag="wtps")
            nc.tensor.matmul(ps2[:], lhsT=bit[:], rhs=wT)
            cst = pp.tile([1, 128], FP, tag="cstrow")
            nc.scalar.copy(cst[:], ps2[:])
            nc.sync.dma_start(dst[3:4, :], cst[:])

        # ---------- u rows -> DRAM ; v -> SBUF ----------
        # v stored pair-interleaved: chunk g (32 n-rows) lives at partition
        # base (g%2)*32, column block g//2 -- matmul lhsT/rhs base partitions
        # can only be 0/32/64.
        vctx = ExitStack()
        vpool = vctx.enter_context(tc.tile_pool(name="vpool", bufs=1, side="right"))
        v2 = vpool.tile([64, 32 * 128], FR, tag="v2")
        with tc.tile_pool(name="uvp", bufs=3) as uvp, \
                tc.tile_pool(name="uvps", bufs=3, space="PSUM") as uvpsum:
            for i in range(NTILES):
                pu = uvpsum.tile([128, 128], FP, tag="pu")
                nc.tensor.matmul(pu[:], lhsT=fl[:, _ts(i, 128)], rhs=ur[:])
                su = uvp.tile([128, 128], FP, tag="su")
                nc.scalar.copy(su[:], pu[:])
                nc.sync.dma_start(u_rows[_ts(i, 128), :], su[:])
                if dbg:
                    nc.sync.dma_start(dbg["urows"][_ts(i, 128), :], su[:])
            for m in range(32):
                pv = uvpsum.tile([64, 128], FP, tag="pv")
                for half in range(2):
                    g = 2 * m + half
                    nc.tensor.matmul(pv[_ts(half, 32), :],
                                     lhsT=fl[:, _ts(g, 32)], rhs=vr[:],
                                     skip_group_check=True)
                nc.scalar.copy(v2[:, _ts(m, 128)], pv[:])

        # ---------- knn ----------
        # 128-partition layout: dma_gather reads its index operand as a
        # (128, num_idxs/16) view using only the first 16 partitions, but
        # bounds-checks all of it -- zero the rest.
        idxT = wp.tile([128, N], I16, tag="idxT")
        nc.sync.dma_start(idxT[:], zeros16_t[:])
        with tc.tile_pool(name="knn", bufs=2) as kp, \
                tc.tile_pool(name="knnps", bufs=1, space="PSUM") as kps, \
                tc.tile_pool(name="knnpt", bufs=2, space="PSUM") as kpt, \
                tc.tile_pool(name="knnsm", bufs=2) as ksm:
            for t in range(NTILES):
                dp = kps.tile([128, N], FP, tag="dp")
                for j in range(NCHUNK):
                    nc.tensor.matmul(
                        dp[:, _ts(j, 512)],
                        lhsT=lmat[:, _ts(t, 128)],
                        rhs=rmat[:, _ts(j, 512)],
                    )
                keys = kp.tile([128, N], FP, tag="keys")
                ki = keys[:].bitcast(I32)
                # key = bits(D+1) ^ 0x7fffffff: positive normal floats,
                # strictly decreasing in D (the +1 rides in the matmul)
                nc.vector.tensor_scalar(
                    out=ki, in0=dp[:].bitcast(I32),
                    scalar1=0x7FFFFFFF, scalar2=None,
                    op0=ALU.bitwise_xor,
                )
                m16 = ksm.tile([128, 16], FP, tag="m16")
                idx16 = ksm.tile([128, 16], mybir.dt.uint16, tag="idx16")
                nc.vector.max(m16[:, 0:8], keys[:])
                nc.vector.max_index(idx16[:, 0:8], m16[:, 0:8], keys[:])
                nc.vector.match_replace(out=keys[:], in_to_replace=m16[:, 0:8],
                                        in_values=keys[:], imm_value=0.0)
                nc.vector.max(m16[:, 8:16], keys[:])
                nc.vector.max_index(idx16[:, 8:16], m16[:, 8:16], keys[:])
                # transpose (128,16) -> (16,128): go through fp32 VALUES
                # (0..2047 exact) -- int16 is not a legal PE transpose dtype
                idxf = ksm.tile([128, 16], FP, tag="idxf")
                nc.vector.tensor_copy(idxf[:], idx16[:])
                pt = kpt.tile([16, 128], FP, tag="pt")
                nc.tensor.matmul(pt[:], lhsT=idxf[:], rhs=ident[:],
                                 is_transpose=True, skip_group_check=True)
                nc.vector.tensor_copy(idxT[0:16, _ts(t, 128)], pt[:])
                # dma_gather reads its index operand per-Q7-core (each of the
                # 8 cores reads its own 16-partition group) -- replicate this
                # tile's columns across all 128 partitions right away so the
                # pass-1 gathers can start while knn continues.
                for r in range(7):
                    nc.sync.dma_start(idxT[16 * (r + 1):16 * (r + 2), _ts(t, 128)],
                                      idxT[0:16, _ts(t, 128)])

        # ---------- pass 1: y = uT + v, stats ----------
        pctx.close()  # frees prep/knn-era SBUF before the big y buffer
        bigp = ctx.enter_context(tc.tile_pool(name="bigp", bufs=1))
        y_all = bigp.tile([128, NEDGE], FP, tag="y_all")
        ysum_c = wp.tile([128, ECHUNKS], FP, tag="ysum_c")
        ysq_c = wp.tile([128, ECHUNKS], FP, tag="ysq_c")
        with tc.tile_pool(name="gat", bufs=3) as gp, \
                tc.tile_pool(name="yps", bufs=4, space="PSUM") as yps, \
                tc.tile_pool(name="ysq", bufs=1) as ysqp:
            for gt in range(NTILES * 2):
                ug = gp.tile([128, 8, 128], FP, tag="ug")
                nc.gpsimd.dma_gather(
                    out_ap=ug[:], in_ap=u_rows[:],
                    idxs_ap=idxT[:, _ts(gt, 64)],
                    num_idxs=1024, num_idxs_reg=1024, elem_size=128,
                )
                if dbg and gt == 0:
                    nc.sync.dma_start(dbg["ug0"][:, :], ug[:].rearrange("p a b -> p (a b)"))
                for j in range(2):
                    c = gt * 2 + j
                    yp = yps.tile([128, 512], FP, tag="yp")
                    for q in range(4):
                        # start=True only on the first op: a start re-arms the
                        # whole 2KB lazy-zero region, clobbering sibling
                        # quarters already written.
                        nc.tensor.matmul(
                            yp[:, _ts(q, 128)],
                            lhsT=ug[:, j * 4 + q, :],
                            rhs=ident[:],
                            is_transpose=True, start=(q == 0), stop=False,
                            skip_group_check=True,
                        )
                    nc.tensor.matmul(
                        yp[:],
                        lhsT=v2[_ts(c % 2, 32), _ts(c // 2, 128)],
                        rhs=b32[_ts(c % 2, 32), :],
                        start=False, stop=True, skip_group_check=True,
                    )
                    nc.scalar.activation(
                        out=y_all[:, _ts(c, 512)], in_=yp[:], func=ACTF.Copy,
                        accum_out=ysum_c[:, c:c + 1],
                    )
                    sqs = ysqp.tile([128, 512], FP, tag="sqs")
                    nc.scalar.activation(
                        out=sqs[:], in_=yp[:], func=ACTF.Square,
                        accum_out=ysq_c[:, c:c + 1],
                    )

        if dbg:
            nc.sync.dma_start(dbg["idxT"][:, :], idxT[0:16, :])
            nc.sync.dma_start(dbg["ysum"][:, :], ysum_c[:])
            nc.sync.dma_start(dbg["ysq"][:, :], ysq_c[:])
            nc.sync.dma_start(dbg["y0"][:, :], y_all[:, 0:512])
        vctx.close()  # v2 dead after pass 1

        # ---------- group norm scale/bias ----------
        sc_sb = wp.tile([128, 1], FP, tag="sc_sb")
        tb_sb = wp.tile([128, 1], FP, tag="tb_sb")
        qs_sb = wp.tile([128, 1], FP, tag="qs_sb")
        qb_sb = wp.tile([128, 1], FP, tag="qb_sb")
        with tc.tile_pool(name="st", bufs=1) as stp, \
                tc.tile_pool(name="stps", bufs=1, space="PSUM") as stps:
            stat2 = stp.tile([128, 2], FP, tag="stat2")
            nc.vector.reduce_sum(stat2[:, 0:1], ysum_c[:], axis=AX.X)
            nc.vector.reduce_sum(stat2[:, 1:2], ysq_c[:], axis=AX.X)
            gps = stps.tile([GROUPS, 2], FP, tag="gps")
            nc.tensor.matmul(gps[:], lhsT=gi[:], rhs=stat2[:])
            cnt = float(NEDGE * 32)
            g2s = stp.tile([GROUPS, 2], FP, tag="g2s")
            nc.scalar.mul(g2s[:], gps[:], 1.0 / cnt)  # [mean, E[y^2]]
            rm = stp.tile([GROUPS, 2], FP, tag="rm")
            # rm[:,1] = mu ; rm[:,0] = rstd
            nc.scalar.copy(rm[:, 1:2], g2s[:, 0:1])
            musq = stp.tile([GROUPS, 1], FP, tag="musq")
            nc.scalar.square(musq[:], g2s[:, 0:1])
            var = stp.tile([GROUPS, 1], FP, tag="var")
            nc.vector.tensor_tensor(out=var[:], in0=g2s[:, 1:2], in1=musq[:],
                                    op=ALU.subtract)
            vpe = stp.tile([GROUPS, 1], FP, tag="vpe")
            nc.vector.tensor_scalar(out=vpe[:], in0=var[:], scalar1=EPS,
                                    scalar2=None, op0=ALU.add)
            vin = stp.tile([GROUPS, 1], FP, tag="vin")
            nc.vector.reciprocal(vin[:], vpe[:])
            nc.scalar.sqrt(rm[:, 0:1], vin[:])
            bc2p = stps.tile([128, 2], FP, tag="bc2p")
            nc.tensor.matmul(bc2p[:], lhsT=git[:], rhs=rm[:])
            bc2 = stp.tile([128, 2], FP, tag="bc2")
            nc.scalar.copy(bc2[:], bc2p[:])
            # sc = gn_g * rstd_c ; tb = gn_b - mu_c * sc
            nc.vector.tensor_tensor(out=sc_sb[:], in0=gng[:], in1=bc2[:, 0:1],
                                    op=ALU.mult)
            tmp = stp.tile([128, 1], FP, tag="tmp")
            nc.vector.tensor_tensor(out=tmp[:], in0=bc2[:, 1:2], in1=sc_sb[:],
                                    op=ALU.mult)
            nc.vector.tensor_tensor(out=tb_sb[:], in0=gnb[:], in1=tmp[:],
                                    op=ALU.subtract)
            nc.scalar.mul(qs_sb[:], sc_sb[:], -0.8)
            nc.scalar.mul(qb_sb[:], tb_sb[:], -0.8)

        # ---------- pass 2: ynorm -> g1 -> g2 -> maxes ----------
        # local_base comes from raw y (affine+leakyrelu commute with max_k
        # for gamma > 0), applied once at the end -- keeps the local path in
        # exact fp32 while the global path rides fp32r matmuls.
        lbraw = wp.tile([128, N], FP, tag="lbraw")
        lb = wp.tile([128, N], FP, tag="lb")
        gmc = wp.tile([128, 4 * ECHUNKS], FP, tag="gmc")  # 4 h-blocks of 64
        with tc.tile_pool(name="p2", bufs=2) as p2, \
                tc.tile_pool(name="g1ps", bufs=2, space="PSUM") as g1ps, \
                tc.tile_pool(name="g2ps", bufs=1, space="PSUM") as g2ps:
            for c in range(ECHUNKS):
                ysl = y_all[:, _ts(c, 512)]
                nc.vector.tensor_reduce(
                    out=lbraw[:, _ts(c, 32)],
                    in_=ysl.rearrange("p (n k) -> p n k", k=16),
                    op=ALU.max, axis=AX.X,
                )
                z = p2.tile([128, 512], FP, tag="z")
                nc.vector.tensor_scalar(out=z[:], in0=ysl, scalar1=sc_sb[:],
                                        scalar2=tb_sb[:], op0=ALU.mult,
                                        op1=ALU.add)
                q = p2.tile([128, 512], FP, tag="q")
                nc.scalar.activation(out=q[:], in_=ysl, func=ACTF.Relu,
                                     bias=qb_sb[:], scale=qs_sb[:])
                yn = p2.tile([128, 512], FR, tag="yn")
                nc.vector.tensor_tensor(out=yn[:], in0=z[:], in1=q[:],
                                        op=ALU.add)
                g1p = g1ps.tile([128, 1024], FP, tag="g1p")
                for h in range(2):
                    nc.tensor.matmul(g1p[:, _ts(h, 512)],
                                     lhsT=waT[:, _ts(h, 128)],
                                     rhs=yn[:])
                g1r = p2.tile([128, 1024], FR, tag="g1r")
                for h in range(2):
                    nc.scalar.activation(out=g1r[:, _ts(h, 512)],
                                         in_=g1p[:, _ts(h, 512)],
                                         func=ACTF.Relu, bias=ba_c[:, h:h + 1])
                g2p = g2ps.tile([128, 2048], FP, tag="g2p")
                for h in range(4):
                    for kk in range(2):
                        nc.tensor.matmul(
                            g2p[:, _ts(h, 512)],
                            lhsT=wbT[kk][:, _ts(h, 128)],
                            rhs=g1r[:, _ts(kk, 512)],
                            start=(kk == 0), stop=(kk == 1),
                        )
                # split g2 consumption: DVE reads PSUM at 1 elem/cycle, so
                # move half the banks to SBUF via DMA and reduce on GPSIMD
                for h in range(2):
                    nc.vector.tensor_reduce(
                        out=gmc[:, h * ECHUNKS + c:h * ECHUNKS + c + 1],
                        in_=g2p[:, _ts(h, 512)], op=ALU.max, axis=AX.X)
                g2sb = p2.tile([128, 1024], FP, tag="g2sb")
                nc.scalar.copy(g2sb[:], g2p[:, 1024:2048])
                for h in (2, 3):
                    nc.vector.tensor_reduce(
                        out=gmc[:, h * ECHUNKS + c:h * ECHUNKS + c + 1],
                        in_=g2sb[:, _ts(h - 2, 512)], op=ALU.max, axis=AX.X)

            # lb = leakyrelu(sc * lbraw + tb) = z + relu(-0.8 z), chunked
            for j in range(NCHUNK):
                lsl = lbraw[:, _ts(j, 512)]
                z2 = p2.tile([128, 512], FP, tag="z")
                nc.vector.tensor_scalar(out=z2[:], in0=lsl, scalar1=sc_sb[:],
                                        scalar2=tb_sb[:], op0=ALU.mult,
                                        op1=ALU.add)
                q2 = p2.tile([128, 512], FP, tag="q")
                nc.scalar.activation(out=q2[:], in_=lsl, func=ACTF.Relu,
                                     bias=qb_sb[:], scale=qs_sb[:])
                nc.vector.tensor_tensor(out=lb[:, _ts(j, 512)], in0=z2[:],
                                        in1=q2[:], op=ALU.add)

        if dbg:
            nc.sync.dma_start(dbg["lbraw"][:, :], lbraw[:])
            nc.sync.dma_start(dbg["lb"][:, :], lb[:])

        # ---------- gmax -> relu(+bb) ----------
        gmaxr = wp.tile([128, 4], FP, tag="gmaxr")
        with tc.tile_pool(name="gm", bufs=1) as gmp:
            for h in range(4):
                gm1 = gmp.tile([128, 1], FP, tag="gm1")
                nc.vector.tensor_reduce(out=gm1[:],
                                        in_=gmc[:, _ts(h, ECHUNKS)],
                                        op=ALU.max, axis=AX.X)
                nc.scalar.activation(out=gmaxr[:, h:h + 1], in_=gm1[:],
                                     func=ACTF.Relu, bias=bb_c[:, h:h + 1])

        if dbg:
            nc.sync.dma_start(dbg["gmaxr"][:, :], gmaxr[:])

        # ---------- tail: mlp2 / mlp3 ----------
        lf_sb = wp.tile([128, N], FP, tag="lf_sb")
        glcol = wp.tile([128, 4 * NCHUNK], FP, tag="glcol")
        with tc.tile_pool(name="tl", bufs=2) as tl, \
                tc.tile_pool(name="tlps", bufs=1, space="PSUM") as tlps, \
                tc.tile_pool(name="tlps2", bufs=1, space="PSUM") as tlps2:
            # wbias = bc + Wc[:,128:] @ gmaxr
            wbias = wp.tile([128, 2], FP, tag="wbias")
            for h in range(2):
                wcp = tlps.tile([128, 1], FP, tag="wcp")
                for k in range(1, 5):
                    nc.tensor.matmul(wcp[:],
                                     lhsT=wcT[k][:, _ts(h, 128)],
                                     rhs=gmaxr[:, k - 1:k],
                                     start=(k == 1), stop=(k == 4))
                nc.vector.tensor_tensor(out=wbias[:, h:h + 1], in0=wcp[:],
                                        in1=bc_c[:, h:h + 1], op=ALU.add)

            for j in range(NCHUNK):
                lf1p = tlps.tile([128, 1024], FP, tag="sh1024")
                for h in range(2):
                    nc.tensor.matmul(lf1p[:, _ts(h, 512)],
                                     lhsT=wcT[0][:, _ts(h, 128)],
                                     rhs=lb[:, _ts(j, 512)])
                lf1r = tl.tile([128, 1024], FR, tag="lf1r")
                for h in range(2):
                    nc.scalar.activation(out=lf1r[:, _ts(h, 512)],
                                         in_=lf1p[:, _ts(h, 512)],
                                         func=ACTF.Relu, bias=wbias[:, h:h + 1])
                lf2p = tlps2.tile([128, 512], FP, tag="lf2p")
                for kk in range(2):
                    nc.tensor.matmul(lf2p[:], lhsT=wdT[kk][:],
                                     rhs=lf1r[:, _ts(kk, 512)],
                                     start=(kk == 0), stop=(kk == 1))
                nc.scalar.activation(out=lf_sb[:, _ts(j, 512)], in_=lf2p[:],
                                     func=ACTF.Relu, bias=bd_c[:])
                gl1p = tlps.tile([128, 1024], FP, tag="sh1024")
                for h in range(2):
                    nc.tensor.matmul(gl1p[:, _ts(h, 512)],
                                     lhsT=weT[:, _ts(h, 128)],
                                     rhs=lf_sb[:, _ts(j, 512)])
                gl1r = tl.tile([128, 1024], FR, tag="gl1r")
                for h in range(2):
                    nc.scalar.activation(out=gl1r[:, _ts(h, 512)],
                                         in_=gl1p[:, _ts(h, 512)],
                                         func=ACTF.Relu, bias=be_c[:, h:h + 1])
                gl2p = tlps2.tile([128, 2048], FP, tag="gl2p")
                for h in range(4):
                    for kk in range(2):
                        nc.tensor.matmul(
                            gl2p[:, _ts(h, 512)],
                            lhsT=wfT[kk][:, _ts(h, 128)],
                            rhs=gl1r[:, _ts(kk, 512)],
                            start=(kk == 0), stop=(kk == 1))
                for h in range(4):
                    nc.vector.tensor_reduce(
                        out=glcol[:, h * NCHUNK + j:h * NCHUNK + j + 1],
                        in_=gl2p[:, _ts(h, 512)], op=ALU.max, axis=AX.X)

            nc.sync.dma_start(loc_out[:, :], lf_sb[:])
            for h in range(4):
                gf = tl.tile([128, 1], FP, tag="gf")
                nc.vector.tensor_reduce(out=gf[:], in_=glcol[:, _ts(h, NCHUNK)],
                                        op=ALU.max, axis=AX.X)
                gfr = tl.tile([128, 1], FP, tag="gfr")
                nc.scalar.activation(out=gfr[:], in_=gf[:], func=ACTF.Relu,
                                     bias=bf_c[:, h:h + 1])
                nc.sync.dma_start(glob_out[_ts(h, 128), :], gfr[:])


_NC_CACHE = None


def _get_program():
    global _NC_CACHE
    if _NC_CACHE is None:
        _NC_CACHE = build_program()
    return _NC_CACHE


def _make_in_maps(inputs):
    B = inputs["x"].shape[0]
    in_maps = []
    for b in range(B):
        m = {}
        for name, shape in INPUT_SPECS:
            arr = np.asarray(inputs[name], dtype=np.float32)
            if name in ("x", "f"):
                arr = arr[b]
            m[name] = np.ascontiguousarray(arr)
        in_maps.append(m)
    return in_maps


def kernel(**inputs):
    nc = _get_program()
    B = inputs["x"].shape[0]
    assert B == 8
    in_maps = _make_in_maps(inputs)
    res = run_bass_kernel_spmd(nc, in_maps, list(range(B)))
    glob = np.stack([res.results[b]["glob"] for b in range(B)])  # (B,512,1)
    loc = np.stack([res.results[b]["loc"] for b in range(B)])    # (B,128,N)
    return (glob.astype(np.float32), loc.astype(np.float32))


if __name__ == "__main__":
    prog = build_program()
    print("program built OK")


# revision 45
# speedup vs baseline: 488.4010x; 1.0770x over previous
"""DGCNN_Grouper (knn + edgeconv + mlps) Trainium2 Bass kernel.

Strategy: batch-parallel over 8 NeuronCores (B=8 -> 1 sample/core).
Per core (N=2048, K=16):
  - knn: D+1 computed as ONE augmented K=6 matmul per (128,512) chunk:
    D+1 = [x,1,sq,1]^T . [-2x, sq, 1, 1]. Top-16-smallest per row via
    keys = bits(D+1)^0x7fffffff (positive floats, descending in D) with
    vector max8 / max_index / match_replace / max8 / max_index --
    exact fp32 set selection.
  - edge features:  y[n,k,:] = u[idx[n,k]] + v[n]  where
       u = h @ W1a^T, v = h @ (W1b-W1a)^T, h = f^T W_it^T + b_it
    (u,v folded to direct-from-f matmuls). u gathered row-wise from DRAM
    with the gpsimd dma_gather custom op, PE-transposed into
    (channel, edge) layout, v added via an accumulated 0/1 matmul.
  - GroupNorm stats from per-channel accum_out sums; affine+leakyrelu as
    z + relu(-0.8 z) with scale/bias folded into the ACT op.
  - mlp1/maxes fully on chip, global maxes folded with relu/bias.
  - mlp2/mlp3 on (n) with the gmax contribution folded into the bias.
"""

import os
import sys

import numpy as np

for _p in ("/opt/trn_rl_repo", "/root/.axon_site/_ro/trn_rl_repo"):
    if os.path.isdir(_p) and _p not in sys.path:
        sys.path.insert(0, _p)

import concourse.bacc as bacc
import concourse.bass as bass
import concourse.tile as tile
from concourse import library_config, mybir
from concourse.bass_utils import run_bass_kernel_spmd

FP = mybir.dt.float32
FR = mybir.dt.float32r if os.environ.get("USE_F32R", "1") == "1" else mybir.dt.float32
I32 = mybir.dt.int32
I16 = mybir.dt.int16

N = 2048
K = 16
GROUPS = 4
EPS = 1e-5
NCHUNK = N // 512  # 4
NTILES = N // 128  # 16
NEDGE = N * K  # 32768
ECHUNKS = NEDGE // 512  # 64

AX = mybir.AxisListType
ALU = mybir.AluOpType
ACTF = mybir.ActivationFunctionType

INPUT_SPECS = [
    ("x", (3, N)), ("f", (3, N)),
    ("W_it", (64, 3)), ("b_it", (64,)),
    ("W1", (128, 128)), ("gn_g", (128,)), ("gn_b", (128,)),
    ("Wa", (256, 128)), ("ba", (256,)),
    ("Wb", (512, 256)), ("bb", (512,)),
    ("Wc", (256, 640)), ("bc", (256,)),
    ("Wd", (128, 256)), ("bd", (128,)),
    ("We", (256, 128)), ("be", (256,)),
    ("Wf", (512, 256)), ("bf", (512,)),
]


def _ts(i, sz):
    return slice(i * sz, (i + 1) * sz)


def build_program():
    nc = bacc.Bacc("TRN2", target_bir_lowering=False, debug=False)

    ins = {}
    for name, shape in INPUT_SPECS:
        ins[name] = nc.dram_tensor(name, list(shape), FP, kind="ExternalInput").ap()
    glob_out = nc.dram_tensor("glob", [512, 1], FP, kind="ExternalOutput").ap()
    loc_out = nc.dram_tensor("loc", [128, N], FP, kind="ExternalOutput").ap()
    u_rows = nc.dram_tensor("u_rows", [N, 128], FP).ap()
    dbg = {}
    if os.environ.get("DEBUG_OUT", "0") == "1":
        dbg["idxT"] = nc.dram_tensor("dbg_idxT", [16, N], I16, kind="ExternalOutput").ap()
        dbg["lbraw"] = nc.dram_tensor("dbg_lbraw", [128, N], FP, kind="ExternalOutput").ap()
        dbg["ysum"] = nc.dram_tensor("dbg_ysum", [128, ECHUNKS], FP, kind="ExternalOutput").ap()
        dbg["ysq"] = nc.dram_tensor("dbg_ysq", [128, ECHUNKS], FP, kind="ExternalOutput").ap()
        dbg["gmaxr"] = nc.dram_tensor("dbg_gmaxr", [128, 4], FP, kind="ExternalOutput").ap()
        dbg["urows"] = nc.dram_tensor("dbg_urows", [N, 128], FP, kind="ExternalOutput").ap()
        dbg["y0"] = nc.dram_tensor("dbg_y0", [128, 512], FP, kind="ExternalOutput").ap()
        dbg["ug0"] = nc.dram_tensor("dbg_ug0", [128, 8 * 128], FP, kind="ExternalOutput").ap()
        dbg["lb"] = nc.dram_tensor("dbg_lb", [128, N], FP, kind="ExternalOutput").ap()

    # constant data baked into the NEFF
    ident_np = np.eye(128, dtype=np.float32)
    ident_t = nc.inline_tensor(ident_np, name="ident").ap()
    ident16_t = nc.inline_tensor(np.eye(128, dtype=np.int16), name="ident16").ap()
    iota_np = np.broadcast_to(np.arange(N, dtype=np.int32), (128, N)).copy()
    iota_t = nc.inline_tensor(iota_np, name="iotat").ap()
    # B pattern repeated every 32 partitions so a v-slice at base partition
    # 32*j can pair with rhs slice b32[32*j:32*(j+1), :] (matmul requires
    # equal base partitions for lhsT and rhs).
    b32_np = np.zeros((128, 512), dtype=np.float32)
    for p in range(128):
        r = p % 32
        b32_np[p, r * 16:(r + 1) * 16] = 1.0
    b32_t = nc.inline_tensor(b32_np, name="b32").ap()
    ones_t_ap = nc.inline_tensor(np.ones((1, N), dtype=np.float32), name="onesrow").ap()
    zeros16_t = nc.inline_tensor(np.zeros((128, N), dtype=np.int16), name="zeros16").ap()
    gi_np = np.zeros((128, GROUPS), dtype=np.float32)
    for g in range(GROUPS):
        gi_np[g * 32:(g + 1) * 32, g] = 1.0
    gi_t = nc.inline_tensor(gi_np, name="gi").ap()
    git_t = nc.inline_tensor(gi_np.T.copy(), name="git").ap()

    with tile.TileContext(nc) as tc:
        _body(nc, tc, ins, glob_out, loc_out, u_rows,
              ident_t, ident16_t, iota_t, b32_t, gi_t, git_t, ones_t_ap,
              zeros16_t, dbg)
    nc.compile()
    return nc


def _body(nc, tc, ins, glob_out, loc_out, u_rows,
          ident_t, ident16_t, iota_t, b32_t, gi_t, git_t, ones_t, zeros16_t,
          dbg=None):
    dbg = dbg or {}
    from contextlib import ExitStack

    ctx = ExitStack()
    with ctx:
        wp = ctx.enter_context(tc.tile_pool(name="wp", bufs=1))
        pctx = ExitStack()
        pp = pctx.enter_context(tc.tile_pool(name="pp", bufs=1))
        wpsum = pctx.enter_context(tc.tile_pool(name="wpsum", bufs=2, space="PSUM"))

        # ---------- constants into SBUF ----------
        ident = wp.tile([128, 128], FP, tag="ident")
        nc.sync.dma_start(ident[:], ident_t[:])
        ident16 = pp.tile([128, 128], I16, tag="ident16")
        nc.sync.dma_start(ident16[:], ident16_t[:])
        b32f = pp.tile([128, 512], FP, tag="b32f")
        nc.sync.dma_start(b32f[:], b32_t[:])
        b32 = wp.tile([128, 512], FR, tag="b32")
        nc.vector.tensor_copy(b32[:], b32f[:])
        gi = wp.tile([128, GROUPS], FP, tag="gi")
        nc.sync.dma_start(gi[:], gi_t[:])
        git = wp.tile([GROUPS, 128], FP, tag="git")
        nc.sync.dma_start(git[:], git_t[:])

        # ---------- load weights + transposes ----------
        def load_nat(name, O, C):
            """W (O,C) -> sbuf tile (128, (O//128)*C); block j holds rows
            j*128..j*128+127."""
            ap = ins[name]
            jb = O // 128
            t = pp.tile([128, jb * C], FP, tag=f"nat_{name}")
            for j in range(jb):
                nc.sync.dma_start(t[:, _ts(j, C)], ap[_ts(j, 128), :])
            return t

        def transpose_weight(name, O, C, dtype=FP):
            """Return list of tiles T[k] (128, O) with T[k][c,o] = W[o, k*128+c].
            dtype=FR makes the DVE psum->sbuf copy emit fp32r (rounded) so the
            tile can feed fp32r matmuls."""
            nat = load_nat(name, O, C)
            jb, kb = O // 128, C // 128
            tiles = []
            for k in range(kb):
                tk = wp.tile([128, O], dtype, tag=f"wT_{name}_{k}")
                for j in range(jb):
                    ps = wpsum.tile([128, 128], FP, tag="wtps")
                    nc.tensor.matmul(
                        ps[:],
                        lhsT=nat[:, _ts(j, C)][:, _ts(k, 128)],
                        rhs=ident[:],
                        is_transpose=True, skip_group_check=True,
                    )
                    nc.vector.tensor_copy(tk[:, _ts(j, 128)], ps[:])
                tiles.append(tk)
            return tiles

        def load_vec_cols(name, D, tag=None):
            """(D,) -> (128, D//128): col c = elems c*128..c*128+127."""
            cb = max(1, D // 128)
            p = min(D, 128)
            t = wp.tile([p, cb], FP, tag=tag or f"vec_{name}")
            ap = ins[name]
            for c in range(cb):
                nc.sync.dma_start(t[:, c:c + 1], ap[_ts(c, p), None])
            return t

        waT = transpose_weight("Wa", 256, 128, FR)[0]   # (128, 256)
        wbT = transpose_weight("Wb", 512, 256, FR)      # 2 x (128, 512)
        wcT = transpose_weight("Wc", 256, 640)          # 5 x (128, 256)
        wdT = transpose_weight("Wd", 128, 256, FR)      # 2 x (128, 128)
        weT = transpose_weight("We", 256, 128)[0]       # (128, 256)
        wfT = transpose_weight("Wf", 512, 256, FR)      # 2 x (128, 512)

        ba_c = load_vec_cols("ba", 256)
        bb_c = load_vec_cols("bb", 512)
        bc_c = load_vec_cols("bc", 256)
        bd_c = load_vec_cols("bd", 128)
        be_c = load_vec_cols("be", 256)
        bf_c = load_vec_cols("bf", 512)
        gng = load_vec_cols("gn_g", 128)
        gnb = load_vec_cols("gn_b", 128)
        bit = load_vec_cols("b_it", 64)  # wp, tiny

        # W1 -> W1T, split a/b, w1bm = W1b - W1a
        w1nat = load_nat("W1", 128, 128)
        w1T = pp.tile([128, 128], FP, tag="w1T")
        ps = wpsum.tile([128, 128], FP, tag="wtps")
        nc.tensor.matmul(ps[:], lhsT=w1nat[:], rhs=ident[:],
                         is_transpose=True, skip_group_check=True)
        nc.vector.tensor_copy(w1T[:], ps[:])
        w1aT = w1T[0:64, :]     # (64, 128)
        # shift W1T rows 64:128 to base partition 0 (DVE needs equal base
        # partitions when both operands are in SBUF)
        w1bT0 = pp.tile([64, 128], FP, tag="w1bT0")
        nc.sync.dma_start(w1bT0[:], w1T[64:128, :])
        w1bmT = pp.tile([64, 128], FP, tag="w1bm")
        nc.vector.tensor_tensor(out=w1bmT[:], in0=w1bT0[:], in1=w1aT,
                                op=ALU.subtract)

        w_it = pp.tile([64, 3], FP, tag="w_it")
        nc.sync.dma_start(w_it[:], ins["W_it"][:, :])

        # ---------- FL = [f; 1]  (4, N) ; x, sq rows ----------
        fl = pp.tile([4, N], FP, tag="fl")
        nc.sync.dma_start(fl[0:3, :], ins["f"][:, :])
        nc.sync.dma_start(fl[3:4, :], ones_t[:, :])

        xs = pp.tile([3, N], FP, tag="xs")
        nc.sync.dma_start(xs[:], ins["x"][:, :])
        xsq = pp.tile([3, N], FP, tag="xsq")
        nc.vector.tensor_tensor(out=xsq[:], in0=xs[:], in1=xs[:], op=ALU.mult)
        ones3 = pp.tile([3, 1], FP, tag="ones3")
        nc.sync.dma_start(ones3[:], ones_t[0:1, 0:3])

        # L = [x,1,sq,1] (6,N) lhsT ; R = [-2x, sq, 1, 1] (6,N) rhs
        lmat = pp.tile([6, N], FP, tag="lmat")
        rmat = pp.tile([6, N], FP, tag="rmat")
        nc.sync.dma_start(lmat[0:3, :], ins["x"][:, :])
        nc.sync.dma_start(lmat[3:4, :], ones_t[:, :])
        nc.sync.dma_start(lmat[5:6, :], ones_t[:, :])
        nc.scalar.mul(rmat[0:3, :], xs[:], -2.0)
        nc.sync.dma_start(rmat[4:5, :], ones_t[:, :])
        nc.sync.dma_start(rmat[5:6, :], ones_t[:, :])
        sqrow = pp.tile([1, N], FP, tag="sqrow")
        for j in range(NCHUNK):
            ps = wpsum.tile([1, 512], FP, tag="wtps")
            nc.tensor.matmul(ps[:], lhsT=ones3[:], rhs=xsq[:, _ts(j, 512)])
            nc.scalar.copy(sqrow[:, _ts(j, 512)], ps[:])
        nc.sync.dma_start(lmat[4:5, :], sqrow[:])
        nc.sync.dma_start(rmat[3:4, :], sqrow[:])

        # ---------- UR / VR (4,128): u = FL^T @ UR, v = FL^T @ VR ----------
        ur = pp.tile([4, 128], FP, tag="ur")
        vr = pp.tile([4, 128], FP, tag="vr")
        for dst, wT in ((ur, w1aT), (vr, w1bmT[:])):
            # rows 0:3 = W_it^T @ wT^T ... comb[i,o] = sum_c W_it[c,i] wT[c,o]
            ps = wpsum.tile([3, 128], FP, tag="wtps")
            nc.tensor.matmul(ps[:], lhsT=w_it[:], rhs=wT)
            nc.scalar.copy(dst[0:3, :], ps[:])
            # row 3 = b_it @ wT
            ps2 = wpsum.tile([1, 128], FP, tag="wtps")
            nc.tensor.matmul(ps2[:], lhsT=bit[:], rhs=wT)
            cst = pp.tile([1, 128], FP, tag="cstrow")
            nc.scalar.copy(cst[:], ps2[:])
            nc.sync.dma_start(dst[3:4, :], cst[:])

        # ---------- u rows -> DRAM ; v -> SBUF ----------
        # v stored pair-interleaved: chunk g (32 n-rows) lives at partition
        # base (g%2)*32, column block g//2 -- matmul lhsT/rhs base partitions
        # can only be 0/32/64.
        vctx = ExitStack()
        vpool = vctx.enter_context(tc.tile_pool(name="vpool", bufs=1, side="right"))
        v2 = vpool.tile([64, 32 * 128], FR, tag="v2")
        with tc.tile_pool(name="uvp", bufs=3) as uvp, \
                tc.tile_pool(name="uvps", bufs=3, space="PSUM") as uvpsum:
            for i in range(NTILES):
                pu = uvpsum.tile([128, 128], FP, tag="pu")
                nc.tensor.matmul(pu[:], lhsT=fl[:, _ts(i, 128)], rhs=ur[:])
                su = uvp.tile([128, 128], FP, tag="su")
                nc.scalar.copy(su[:], pu[:])
                nc.sync.dma_start(u_rows[_ts(i, 128), :], su[:])
                if dbg:
                    nc.sync.dma_start(dbg["urows"][_ts(i, 128), :], su[:])
            for m in range(32):
                pv = uvpsum.tile([64, 128], FP, tag="pv")
                for half in range(2):
                    g = 2 * m + half
                    nc.tensor.matmul(pv[_ts(half, 32), :],
                                     lhsT=fl[:, _ts(g, 32)], rhs=vr[:],
                                     skip_group_check=True)
                nc.scalar.copy(v2[:, _ts(m, 128)], pv[:])

        # ---------- knn ----------
        # 128-partition layout: dma_gather reads its index operand as a
        # (128, num_idxs/16) view using only the first 16 partitions, but
        # bounds-checks all of it -- zero the rest.
        idxT = wp.tile([128, N], I16, tag="idxT")
        nc.sync.dma_start(idxT[:], zeros16_t[:])
        with tc.tile_pool(name="knn", bufs=2) as kp, \
                tc.tile_pool(name="knnps", bufs=1, space="PSUM") as kps, \
                tc.tile_pool(name="knnpt", bufs=2, space="PSUM") as kpt, \
                tc.tile_pool(name="knnsm", bufs=2) as ksm:
            for t in range(NTILES):
                dp = kps.tile([128, N], FP, tag="dp")
                for j in range(NCHUNK):
                    nc.tensor.matmul(
                        dp[:, _ts(j, 512)],
                        lhsT=lmat[:, _ts(t, 128)],
                        rhs=rmat[:, _ts(j, 512)],
                    )
                keys = kp.tile([128, N], FP, tag="keys")
                ki = keys[:].bitcast(I32)
                # key = bits(D+1) ^ 0x7fffffff: positive normal floats,
                # strictly decreasing in D (the +1 rides in the matmul)
                nc.vector.tensor_scalar(
                    out=ki, in0=dp[:].bitcast(I32),
                    scalar1=0x7FFFFFFF, scalar2=None,
                    op0=ALU.bitwise_xor,
                )
                m16 = ksm.tile([128, 16], FP, tag="m16")
                idx16 = ksm.tile([128, 16], mybir.dt.uint16, tag="idx16")
                nc.vector.max(m16[:, 0:8], keys[:])
                nc.vector.max_index(idx16[:, 0:8], m16[:, 0:8], keys[:])
                nc.vector.match_replace(out=keys[:], in_to_replace=m16[:, 0:8],
                                        in_values=keys[:], imm_value=0.0)
                nc.vector.max(m16[:, 8:16], keys[:])
                nc.vector.max_index(idx16[:, 8:16], m16[:, 8:16], keys[:])
                # transpose (128,16) -> (16,128): go through fp32 VALUES
                # (0..2047 exact) -- int16 is not a legal PE transpose dtype
                idxf = ksm.tile([128, 16], FP, tag="idxf")
                nc.vector.tensor_copy(idxf[:], idx16[:])
                pt = kpt.tile([16, 128], FP, tag="pt")
                nc.tensor.matmul(pt[:], lhsT=idxf[:], rhs=ident[:],
                                 is_transpose=True, skip_group_check=True)
                nc.vector.tensor_copy(idxT[0:16, _ts(t, 128)], pt[:])
                # dma_gather reads its index operand per-Q7-core (each of the
                # 8 cores reads its own 16-partition group) -- replicate this
                # tile's columns across all 128 partitions right away so the
                # pass-1 gathers can start while knn continues.
                for r in range(7):
                    nc.sync.dma_start(idxT[16 * (r + 1):16 * (r + 2), _ts(t, 128)],
                                      idxT[0:16, _ts(t, 128)])

        # ---------- pass 1: y = uT + v, stats ----------
        pctx.close()  # frees prep/knn-era SBUF before the big y buffer
        bigp = ctx.enter_context(tc.tile_pool(name="bigp", bufs=1))
        y_all = bigp.tile([128, NEDGE], FP, tag="y_all")
        ysum_c = wp.tile([128, ECHUNKS], FP, tag="ysum_c")
        ysq_c = wp.tile([128, ECHUNKS], FP, tag="ysq_c")
        with tc.tile_pool(name="gat", bufs=3) as gp, \
                tc.tile_pool(name="yps", bufs=4, space="PSUM") as yps, \
                tc.tile_pool(name="ysq", bufs=1) as ysqp:
            for gt in range(NTILES * 2):
                ug = gp.tile([128, 8, 128], FP, tag="ug")
                nc.gpsimd.dma_gather(
                    out_ap=ug[:], in_ap=u_rows[:],
                    idxs_ap=idxT[:, _ts(gt, 64)],
                    num_idxs=1024, num_idxs_reg=1024, elem_size=128,
                )
                if dbg and gt == 0:
                    nc.sync.dma_start(dbg["ug0"][:, :], ug[:].rearrange("p a b -> p (a b)"))
                for j in range(2):
                    c = gt * 2 + j
                    yp = yps.tile([128, 512], FP, tag="yp")
                    for q in range(4):
                        # start=True only on the first op: a start re-arms the
                        # whole 2KB lazy-zero region, clobbering sibling
                        # quarters already written.
                        nc.tensor.matmul(
                            yp[:, _ts(q, 128)],
                            lhsT=ug[:, j * 4 + q, :],
                            rhs=ident[:],
                            is_transpose=True, start=(q == 0), stop=False,
                            skip_group_check=True,
                        )
                    nc.tensor.matmul(
                        yp[:],
                        lhsT=v2[_ts(c % 2, 32), _ts(c // 2, 128)],
                        rhs=b32[_ts(c % 2, 32), :],
                        start=False, stop=True, skip_group_check=True,
                    )
                    nc.scalar.activation(
                        out=y_all[:, _ts(c, 512)], in_=yp[:], func=ACTF.Copy,
                        accum_out=ysum_c[:, c:c + 1],
                    )
                    sqs = ysqp.tile([128, 512], FP, tag="sqs")
                    nc.scalar.activation(
                        out=sqs[:], in_=yp[:], func=ACTF.Square,
                        accum_out=ysq_c[:, c:c + 1],
                    )

        if dbg:
            nc.sync.dma_start(dbg["idxT"][:, :], idxT[0:16, :])
            nc.sync.dma_start(dbg["ysum"][:, :], ysum_c[:])
            nc.sync.dma_start(dbg["ysq"][:, :], ysq_c[:])
            nc.sync.dma_start(dbg["y0"][:, :], y_all[:, 0:512])
        vctx.close()  # v2 dead after pass 1

        # ---------- group norm scale/bias ----------
        sc_sb = wp.tile([128, 1], FP, tag="sc_sb")
        tb_sb = wp.tile([128, 1], FP, tag="tb_sb")
        qs_sb = wp.tile([128, 1], FP, tag="qs_sb")
        qb_sb = wp.tile([128, 1], FP, tag="qb_sb")
        with tc.tile_pool(name="st", bufs=1) as stp, \
                tc.tile_pool(name="stps", bufs=1, space="PSUM") as stps:
            stat2 = stp.tile([128, 2], FP, tag="stat2")
            nc.vector.reduce_sum(stat2[:, 0:1], ysum_c[:], axis=AX.X)
            nc.vector.reduce_sum(stat2[:, 1:2], ysq_c[:], axis=AX.X)
            gps = stps.tile([GROUPS, 2], FP, tag="gps")
            nc.tensor.matmul(gps[:], lhsT=gi[:], rhs=stat2[:])
            cnt = float(NEDGE * 32)
            g2s = stp.tile([GROUPS, 2], FP, tag="g2s")
            nc.scalar.mul(g2s[:], gps[:], 1.0 / cnt)  # [mean, E[y^2]]
            rm = stp.tile([GROUPS, 2], FP, tag="rm")
            # rm[:,1] = mu ; rm[:,0] = rstd
            nc.scalar.copy(rm[:, 1:2], g2s[:, 0:1])
            musq = stp.tile([GROUPS, 1], FP, tag="musq")
            nc.scalar.square(musq[:], g2s[:, 0:1])
            var = stp.tile([GROUPS, 1], FP, tag="var")
            nc.vector.tensor_tensor(out=var[:], in0=g2s[:, 1:2], in1=musq[:],
                                    op=ALU.subtract)
            vpe = stp.tile([GROUPS, 1], FP, tag="vpe")
            nc.vector.tensor_scalar(out=vpe[:], in0=var[:], scalar1=EPS,
                                    scalar2=None, op0=ALU.add)
            vin = stp.tile([GROUPS, 1], FP, tag="vin")
            nc.vector.reciprocal(vin[:], vpe[:])
            nc.scalar.sqrt(rm[:, 0:1], vin[:])
            bc2p = stps.tile([128, 2], FP, tag="bc2p")
            nc.tensor.matmul(bc2p[:], lhsT=git[:], rhs=rm[:])
            bc2 = stp.tile([128, 2], FP, tag="bc2")
            nc.scalar.copy(bc2[:], bc2p[:])
            # sc = gn_g * rstd_c ; tb = gn_b - mu_c * sc
            nc.vector.tensor_tensor(out=sc_sb[:], in0=gng[:], in1=bc2[:, 0:1],
                                    op=ALU.mult)
            tmp = stp.tile([128, 1], FP, tag="tmp")
            nc.vector.tensor_tensor(out=tmp[:], in0=bc2[:, 1:2], in1=sc_sb[:],
                                    op=ALU.mult)
            nc.vector.tensor_tensor(out=tb_sb[:], in0=gnb[:], in1=tmp[:],
                                    op=ALU.subtract)
            nc.scalar.mul(qs_sb[:], sc_sb[:], -0.8)
            nc.scalar.mul(qb_sb[:], tb_sb[:], -0.8)

        # ---------- pass 2: ynorm -> g1 -> g2 -> maxes ----------
        # local_base comes from raw y (affine+leakyrelu commute with max_k
        # for gamma > 0), applied once at the end -- keeps the local path in
        # exact fp32 while the global path rides fp32r matmuls.
        lbraw = wp.tile([128, N], FP, tag="lbraw")
        lb = wp.tile([128, N], FP, tag="lb")
        gmc = wp.tile([128, 4 * ECHUNKS], FP, tag="gmc")  # 4 h-blocks of 64
        with tc.tile_pool(name="p2", bufs=2) as p2, \
                tc.tile_pool(name="g2sbp", bufs=2) as g2sbp, \
                tc.tile_pool(name="g1ps", bufs=2, space="PSUM") as g1ps, \
                tc.tile_pool(name="g2ps", bufs=2, space="PSUM") as g2ps:
            for c in range(ECHUNKS):
                ysl = y_all[:, _ts(c, 512)]
                nc.vector.tensor_reduce(
                    out=lbraw[:, _ts(c, 32)],
                    in_=ysl.rearrange("p (n k) -> p n k", k=16),
                    op=ALU.max, axis=AX.X,
                )
                z = p2.tile([128, 512], FP, tag="z")
                nc.vector.tensor_scalar(out=z[:], in0=ysl, scalar1=sc_sb[:],
                                        scalar2=tb_sb[:], op0=ALU.mult,
                                        op1=ALU.add)
                q = p2.tile([128, 512], FP, tag="q")
                nc.scalar.activation(out=q[:], in_=ysl, func=ACTF.Relu,
                                     bias=qb_sb[:], scale=qs_sb[:])
                yn = p2.tile([128, 512], FR, tag="yn")
                nc.vector.tensor_tensor(out=yn[:], in0=z[:], in1=q[:],
                                        op=ALU.add)
                g1p = g1ps.tile([128, 1024], FP, tag="g1p")
                for h in range(2):
                    nc.tensor.matmul(g1p[:, _ts(h, 512)],
                                     lhsT=waT[:, _ts(h, 128)],
                                     rhs=yn[:])
                g1r = p2.tile([128, 1024], FR, tag="g1r")
                for h in range(2):
                    nc.scalar.activation(out=g1r[:, _ts(h, 512)],
                                         in_=g1p[:, _ts(h, 512)],
                                         func=ACTF.Relu, bias=ba_c[:, h:h + 1])
                # g2 in two half-tiles sharing a bufs=2 slot pair so chunk
                # c+1's first-half matmuls overlap chunk c's consumption.
                # DVE reads PSUM at only 1 elem/cycle, so stage each half to
                # SBUF on ACT and reduce at the 2x two-port SBUF rate.
                for hh in range(2):
                    g2p = g2ps.tile([128, 1024], FP, tag="g2p")
                    for h2 in range(2):
                        h = hh * 2 + h2
                        for kk in range(2):
                            nc.tensor.matmul(
                                g2p[:, _ts(h2, 512)],
                                lhsT=wbT[kk][:, _ts(h, 128)],
                                rhs=g1r[:, _ts(kk, 512)],
                                start=(kk == 0), stop=(kk == 1),
                            )
                    g2sb = g2sbp.tile([128, 1024], FP, tag="g2sb")
                    nc.scalar.copy(g2sb[:], g2p[:])
                    for h2 in range(2):
                        h = hh * 2 + h2
                        nc.vector.tensor_reduce(
                            out=gmc[:, h * ECHUNKS + c:h * ECHUNKS + c + 1],
                            in_=g2sb[:, _ts(h2, 512)], op=ALU.max, axis=AX.X)

            # lb = leakyrelu(sc * lbraw + tb) = z + relu(-0.8 z), chunked
            for j in range(NCHUNK):
                lsl = lbraw[:, _ts(j, 512)]
                z2 = p2.tile([128, 512], FP, tag="z")
                nc.vector.tensor_scalar(out=z2[:], in0=lsl, scalar1=sc_sb[:],
                                        scalar2=tb_sb[:], op0=ALU.mult,
                                        op1=ALU.add)
                q2 = p2.tile([128, 512], FP, tag="q")
                nc.scalar.activation(out=q2[:], in_=lsl, func=ACTF.Relu,
                                     bias=qb_sb[:], scale=qs_sb[:])
                nc.vector.tensor_tensor(out=lb[:, _ts(j, 512)], in0=z2[:],
                                        in1=q2[:], op=ALU.add)

        if dbg:
            nc.sync.dma_start(dbg["lbraw"][:, :], lbraw[:])
            nc.sync.dma_start(dbg["lb"][:, :], lb[:])

        # ---------- gmax -> relu(+bb) ----------
        gmaxr = wp.tile([128, 4], FP, tag="gmaxr")
        with tc.tile_pool(name="gm", bufs=1) as gmp:
            for h in range(4):
                gm1 = gmp.tile([128, 1], FP, tag="gm1")
                nc.vector.tensor_reduce(out=gm1[:],
                                        in_=gmc[:, _ts(h, ECHUNKS)],
                                        op=ALU.max, axis=AX.X)
                nc.scalar.activation(out=gmaxr[:, h:h + 1], in_=gm1[:],
                                     func=ACTF.Relu, bias=bb_c[:, h:h + 1])

        if dbg:
            nc.sync.dma_start(dbg["gmaxr"][:, :], gmaxr[:])

        # ---------- tail: mlp2 / mlp3 ----------
        lf_sb = wp.tile([128, N], FP, tag="lf_sb")
        glcol = wp.tile([128, 4 * NCHUNK], FP, tag="glcol")
        with tc.tile_pool(name="tl", bufs=2) as tl, \
                tc.tile_pool(name="tlps", bufs=1, space="PSUM") as tlps, \
                tc.tile_pool(name="tlps2", bufs=1, space="PSUM") as tlps2:
            # wbias = bc + Wc[:,128:] @ gmaxr
            wbias = wp.tile([128, 2], FP, tag="wbias")
            for h in range(2):
                wcp = tlps.tile([128, 1], FP, tag="wcp")
                for k in range(1, 5):
                    nc.tensor.matmul(wcp[:],
                                     lhsT=wcT[k][:, _ts(h, 128)],
                                     rhs=gmaxr[:, k - 1:k],
                                     start=(k == 1), stop=(k == 4))
                nc.vector.tensor_tensor(out=wbias[:, h:h + 1], in0=wcp[:],
                                        in1=bc_c[:, h:h + 1], op=ALU.add)

            for j in range(NCHUNK):
                lf1p = tlps.tile([128, 1024], FP, tag="sh1024")
                for h in range(2):
                    nc.tensor.matmul(lf1p[:, _ts(h, 512)],
                                     lhsT=wcT[0][:, _ts(h, 128)],
                                     rhs=lb[:, _ts(j, 512)])
                lf1r = tl.tile([128, 1024], FR, tag="lf1r")
                for h in range(2):
                    nc.scalar.activation(out=lf1r[:, _ts(h, 512)],
                                         in_=lf1p[:, _ts(h, 512)],
                                         func=ACTF.Relu, bias=wbias[:, h:h + 1])
                lf2p = tlps2.tile([128, 512], FP, tag="lf2p")
                for kk in range(2):
                    nc.tensor.matmul(lf2p[:], lhsT=wdT[kk][:],
                                     rhs=lf1r[:, _ts(kk, 512)],
                                     start=(kk == 0), stop=(kk == 1))
                nc.scalar.activation(out=lf_sb[:, _ts(j, 512)], in_=lf2p[:],
                                     func=ACTF.Relu, bias=bd_c[:])
                gl1p = tlps.tile([128, 1024], FP, tag="sh1024")
                for h in range(2):
                    nc.tensor.matmul(gl1p[:, _ts(h, 512)],
                                     lhsT=weT[:, _ts(h, 128)],
                                     rhs=lf_sb[:, _ts(j, 512)])
                gl1r = tl.tile([128, 1024], FR, tag="gl1r")
                for h in range(2):
                    nc.scalar.activation(out=gl1r[:, _ts(h, 512)],
                                         in_=gl1p[:, _ts(h, 512)],
                                         func=ACTF.Relu, bias=be_c[:, h:h + 1])
                gl2p = tlps2.tile([128, 2048], FP, tag="gl2p")
                for h in range(4):
                    for kk in range(2):
                        nc.tensor.matmul(
                            gl2p[:, _ts(h, 512)],
                            lhsT=wfT[kk][:, _ts(h, 128)],
                            rhs=gl1r[:, _ts(kk, 512)],
                            start=(kk == 0), stop=(kk == 1))
                for h in range(4):
                    nc.vector.tensor_reduce(
                        out=glcol[:, h * NCHUNK + j:h * NCHUNK + j + 1],
                        in_=gl2p[:, _ts(h, 512)], op=ALU.max, axis=AX.X)

            nc.sync.dma_start(loc_out[:, :], lf_sb[:])
            for h in range(4):
                gf = tl.tile([128, 1], FP, tag="gf")
                nc.vector.tensor_reduce(out=gf[:], in_=glcol[:, _ts(h, NCHUNK)],
                                        op=ALU.max, axis=AX.X)
                gfr = tl.tile([128, 1], FP, tag="gfr")
                nc.scalar.activation(out=gfr[:], in_=gf[:], func=ACTF.Relu,
                                     bias=bf_c[:, h:h + 1])
                nc.sync.dma_start(glob_out[_ts(h, 128), :], gfr[:])


_NC_CACHE = None


def _get_program():
    global _NC_CACHE
    if _NC_CACHE is None:
        _NC_CACHE = build_program()
    return _NC_CACHE


def _make_in_maps(inputs):
    B = inputs["x"].shape[0]
    in_maps = []
    for b in range(B):
        m = {}
        for name, shape in INPUT_SPECS:
            arr = np.asarray(inputs[name], dtype=np.float32)
            if name in ("x", "f"):
                arr = arr[b]
            m[name] = np.ascontiguousarray(arr)
        in_maps.append(m)
    return in_maps


def kernel(**inputs):
    nc = _get_program()
    B = inputs["x"].shape[0]
    assert B == 8
    in_maps = _make_in_maps(inputs)
    res = run_bass_kernel_spmd(nc, in_maps, list(range(B)))
    glob = np.stack([res.results[b]["glob"] for b in range(B)])  # (B,512,1)
    loc = np.stack([res.results[b]["loc"] for b in range(B)])    # (B,128,N)
    return (glob.astype(np.float32), loc.astype(np.float32))


if __name__ == "__main__":
    prog = build_program()
    print("program built OK")
